# revision 3
# baseline (speedup 1.0000x reference)
"""2-layer GAT (DGL GATConv style) on 8 trn2 NeuronCores via Bass/Tile. v2

Design:
  - Edges dst-sorted on host; cores own contiguous equal node ranges
    [c*NPC, (c+1)*NPC) and all edges whose dst falls in range.
  - Layer 1 phase A (replicated): packed table row (f32 container)
    [feat(D bf16) | el(H f32) | er(H f32) | pad] = x @ [W | Wal | War],
    written to tab1 in groups of 8 tiles (one slab load + one table write
    per group). Feature cols h-innermost (packed col j <-> head j%H).
  - er for own nodes lives in SBUF (er_tab[d, t, h], d = dst position in
    tile t). Layer 1 builds it with two small gathers (tabA/tabB halves,
    host-clipped indices) + mask merge; layer 2 captures it for free from
    phase-A PSUM (the layer-2 transform is sharded, tiles align).
  - Phase B (sharded): per dst-tile, gather src rows (768 B each) with
    dma_gather. S one-hots (slot->dst, for segment-sum matmuls) are
    generated in ONE 2x-rate DVE op per tile (layout [P, P, K] keeps every
    operand innermost-packed); M one-hots (dst->slot transposes, for the
    er broadcast) are host-built fp8 data streamed from DRAM (0/1 is exact
    in fp8; half the bytes of bf16). er per slot = K tiny M^T @ er_tab
    matmuls instead of a 256 B/edge gather. exp(leakyrelu(el+er)) ->
    weighted messages -> segment sum via S matmuls into PSUM [num | den];
    normalize, bias, relu.
  - Layer 2 phase A is fused into phase B of layer 1: each B0 tile's
    output hb is PE-transposed and immediately transformed (2 matmuls) to
    its packed table row; rows go to tab2loc [NPC, ROWF]. After B0 an
    AllGather concatenates the 8 slices into the full tab2 - no hT
    roundtrip and no replicated layer-2 transform.
"""

import math
import sys
from types import SimpleNamespace

import numpy as np

sys.path.insert(0, "/opt/trn_rl_repo")

from concourse import bacc, bass, mybir, tile  # noqa: E402

F32 = mybir.dt.float32
BF16 = mybir.dt.bfloat16
FP8 = mybir.dt.float8e4
I32 = mybir.dt.int32
I16 = mybir.dt.int16

P = 128


def make_cfg(N=50000, E=800000, D=256, H=8, cores=8, split=None):
    HD = D // H
    NPC = N // cores
    NT = math.ceil(NPC / P)          # dst tiles per core
    NTA = math.ceil(N / P)           # phase-A node tiles (layer 1, flat)
    ROWU = ((D + 4 * H + 127) // 128) * 128   # packed row, bf16 units (256B mult)
    if split is None:
        split = NPC * min(cores, 32767 // NPC)
        split = min(split, N)
    assert split % NPC == 0 and split <= 32767 + 1 and N - split <= 32767 + 1
    return SimpleNamespace(
        N=N, E=E, D=D, H=H, HD=HD, cores=cores, NPC=NPC, NT=NT, NTA=NTA,
        ROWU=ROWU, ROWF=ROWU // 2, SPLIT=split,
        G1=8, GB=7,
        NEG=0.2,
    )


# ---------------------------------------------------------------- host prep

def perm_h_inner(D, H):
    """perm[j] = original feature index stored at packed col j (h-innermost)."""
    HD = D // H
    j = np.arange(D)
    return (j % H) * HD + j // H


def attn_cols(W, a, H):
    D = W.shape[0]
    HD = W.shape[1] // H
    return np.stack(
        [W[:, h * HD:(h + 1) * HD] @ a[h] for h in range(H)], axis=1
    )


def wrap16(flat, reps=8):
    """[num] -> [16*reps, num//16] int16 wrapped layout (idx i at [i%16, i//16]),
    replicated across the 8 gpsimd cores."""
    num = len(flat)
    assert num % 16 == 0
    a = np.zeros((16, num // 16), dtype=np.int16)
    a[np.arange(num) % 16, np.arange(num) // 16] = flat
    return np.tile(a, (reps, 1))


def prep_edges(src, dst, cfg):
    c = cfg
    order = np.argsort(dst, kind="stable")
    src_s = src[order].astype(np.int64)
    dst_s = dst[order].astype(np.int64)

    core = dst_s // c.NPC
    loc = dst_s - core * c.NPC
    lt = loc // P
    pos = loc - lt * P
    islow = src_s < c.SPLIT

    # group = (core, tile, islow) ; edges of a group get consecutive slots
    g = (core * c.NT + lt) * 2 + (1 - islow)   # low first
    order2 = np.argsort(g, kind="stable")
    src_s, dst_s, core, lt, pos, islow, g = (
        a[order2] for a in (src_s, dst_s, core, lt, pos, islow, g))

    uniq, starts = np.unique(g, return_index=True)
    start_of = np.zeros(c.cores * c.NT * 2, dtype=np.int64)
    start_of[uniq] = starts
    rank = np.arange(len(dst_s)) - start_of[g]

    counts = np.zeros(c.cores * c.NT * 2, dtype=np.int64)
    np.add.at(counts, g, 1)
    KA = int(math.ceil(counts[0::2].max() / P))
    KB = int(math.ceil(counts[1::2].max() / P)) if counts[1::2].max() > 0 else 0
    K = KA + KB

    # slot within tile: low edges fill chunks [0,KA), high fill [KA,K)
    chunk = rank // P + np.where(islow, 0, KA)
    part = rank % P

    srcA = np.zeros((c.cores, c.NT * KA * P), dtype=np.int64)
    srcB = np.zeros((c.cores, c.NT * KB * P), dtype=np.int64) if KB else None
    dstpos = np.full((c.cores, P, c.NT * K), 255.0, dtype=np.float32)
    dstposT = np.full((c.cores, c.NT * K * P), 255, dtype=np.int16)

    iA = lt * (KA * P) + (chunk * P + part)            # valid where islow
    iB = lt * (KB * P) + ((chunk - KA) * P + part) if KB else None
    low = islow
    srcA[core[low], iA[low]] = src_s[low]
    if KB:
        hi = ~islow
        srcB[core[hi], iB[hi]] = src_s[hi] - c.SPLIT
    dstpos[core, part, lt * K + chunk] = pos
    dstposT[core, (lt * K + chunk) * P + part] = pos

    # host-built transposed one-hots M[d, (t, ck, p)] = (dstposT == d).
    # fp8e4m3 is exact for 0/1 and halves the stream bytes.
    import ml_dtypes
    Mhot = (dstposT[:, None, :] == np.arange(P, dtype=np.int16)[None, :, None]
            ).astype(ml_dtypes.float8_e4m3fn)

    srcA16 = np.stack([wrap16(srcA[ci]) for ci in range(c.cores)])
    srcB16 = (np.stack([wrap16(srcB[ci]) for ci in range(c.cores)])
              if KB else np.zeros((c.cores, P, 0), np.int16))
    return srcA16, srcB16, dstpos, Mhot, KA, KB


def prep_all(inputs, cfg):
    c = cfg
    perm = perm_h_inner(c.D, c.H)
    x = np.asarray(inputs["data"], np.float32)
    src = np.asarray(inputs["src"]).astype(np.int64)
    dst = np.asarray(inputs["dst"]).astype(np.int64)

    def rhs_for(W, al, ar, permute_rows):
        W = np.asarray(W, np.float64)
        Wal = attn_cols(W, np.asarray(al, np.float64), c.H)
        War = attn_cols(W, np.asarray(ar, np.float64), c.H)
        Wp = W[:, perm]
        if permute_rows:
            Wp, Wal, War = Wp[perm], Wal[perm], War[perm]
        return to_bf16(np.concatenate([Wp, Wal, War], axis=1))

    rhs1 = rhs_for(inputs["W1"], inputs["al1"], inputs["ar1"], False)
    rhs2 = rhs_for(inputs["W2"], inputs["al2"], inputs["ar2"], True)
    b1 = np.asarray(inputs["b1"], np.float32)[perm].reshape(1, c.D)
    b2 = np.asarray(inputs["b2"], np.float32)[perm].reshape(1, c.D)
    xT = to_bf16(x.T.copy())

    srcA16, srcB16, dstpos, Mhot, KA, KB = prep_edges(src, dst, c)

    # er build indices (layer 1): row i (i in [0, NT*128)) <- own node NPC*ci+i
    erbA, erbB, masks = [], [], []
    for ci in range(c.cores):
        base = ci * c.NPC
        rows = np.arange(c.NT * P)
        ra = np.where(base < c.SPLIT, base + rows, 0)
        ra = np.clip(ra, 0, c.SPLIT - 1)
        rb = np.where(base >= c.SPLIT, base - c.SPLIT + rows, 0)
        rb = np.clip(rb, 0, max(c.N - c.SPLIT - 1, 0))
        erbA.append(wrap16(ra))
        erbB.append(wrap16(rb))
        mA = 1.0 if base < c.SPLIT else 0.0
        m = np.zeros((P, 2), np.float32)
        m[:, 0] = mA
        m[:, 1] = 1.0 - mA
        masks.append(m)

    meta = SimpleNamespace(perm=perm, KA=KA, KB=KB, K=KA + KB)
    in_maps = []
    for ci in range(c.cores):
        in_maps.append({
            "xT": xT, "rhs1": rhs1, "rhs2": rhs2, "b1row": b1, "b2row": b2,
            "srcA16": srcA16[ci], "srcB16": srcB16[ci],
            "dstpos": to_bf16(dstpos[ci]),
            "Mhot": Mhot[ci],
            "erbA16": erbA[ci], "erbB16": erbB[ci],
            "maskAB": masks[ci],
        })
    return in_maps, meta


def to_bf16(a):
    import ml_dtypes
    return np.asarray(a).astype(ml_dtypes.bfloat16)


def finalize(results, cfg, meta):
    c = cfg
    parts = [results[ci]["out"][: c.NPC] for ci in range(c.cores)]
    out_p = np.concatenate(parts, axis=0)
    out = np.empty_like(out_p)
    out[:, meta.perm] = out_p
    return out


# ---------------------------------------------------------------- kernel

def build_nc(cfg, KA, KB, debug=False, reps=1):
    c = cfg
    K = KA + KB
    ERR = c.NT * P                   # er rows per core (padded)
    NPAD = c.NTA * P                 # padded table rows (layer 1)
    DH = c.D + c.H                   # seg-matmul rhs cols (msg + den)
    D2H = c.D + 2 * c.H              # phase-A rhs cols
    WC = c.D // 2 + 2 * c.H          # written f32 cols of a packed row

    nc = bacc.Bacc("TRN2", target_bir_lowering=False, debug=debug,
                   num_devices=c.cores, num_swdge_queues=2)

    xT = nc.declare_dram_parameter("xT", [c.D, c.N], BF16, isOutput=False)
    rhs1 = nc.declare_dram_parameter("rhs1", [c.D, D2H], BF16, isOutput=False)
    rhs2 = nc.declare_dram_parameter("rhs2", [c.D, D2H], BF16, isOutput=False)
    b1row = nc.declare_dram_parameter("b1row", [1, c.D], F32, isOutput=False)
    b2row = nc.declare_dram_parameter("b2row", [1, c.D], F32, isOutput=False)
    srcA16 = nc.declare_dram_parameter("srcA16", [P, c.NT * KA * 8], I16, isOutput=False)
    srcB16 = nc.declare_dram_parameter("srcB16", [P, c.NT * KB * 8], I16, isOutput=False)
    dstpos = nc.declare_dram_parameter("dstpos", [P, c.NT * K], BF16, isOutput=False)
    Mhot = nc.declare_dram_parameter("Mhot", [P, c.NT * K * P], FP8, isOutput=False)
    erbA16 = nc.declare_dram_parameter("erbA16", [P, ERR // 16], I16, isOutput=False)
    erbB16 = nc.declare_dram_parameter("erbB16", [P, ERR // 16], I16, isOutput=False)
    maskAB = nc.declare_dram_parameter("maskAB", [P, 2], F32, isOutput=False)
    out_ext = nc.declare_dram_parameter("out", [c.NT * P, c.D], F32, isOutput=True)

    tab1 = nc.dram_tensor("tab1", [NPAD, c.ROWF], F32)
    tab2loc = nc.dram_tensor("tab2loc", [c.NPC, c.ROWF], F32)
    tab2 = nc.dram_tensor("tab2", [c.N, c.ROWF], F32,
                          addr_space="Shared" if c.cores > 4 else "Local")

    with tile.TileContext(nc) as tc:
        with (
            tc.tile_pool(name="const", bufs=1) as constp,
            tc.tile_pool(name="slab", bufs=3) as slabp,
            tc.tile_pool(name="pkg", bufs=2) as pkgp,
            tc.tile_pool(name="eb", bufs=1) as ebp,
            tc.tile_pool(name="ertab", bufs=1) as ertabp,
            tc.tile_pool(name="gath", bufs=4) as gathp,
            tc.tile_pool(name="onehot", bufs=3) as onehotp,
            tc.tile_pool(name="sel", bufs=4) as selp,
            tc.tile_pool(name="rhsm", bufs=2) as rhsmp,
            tc.tile_pool(name="small", bufs=3) as smallp,
            tc.tile_pool(name="lt2", bufs=3) as lt2p,
            tc.tile_pool(name="outp", bufs=2) as outp,
            tc.tile_pool(name="psA", bufs=4, space="PSUM") as psA,
            tc.tile_pool(name="psB", bufs=2, space="PSUM") as psB,
            tc.tile_pool(name="psE", bufs=1, space="PSUM") as psE,
            tc.tile_pool(name="psT", bufs=1, space="PSUM") as psT,
        ):
            # ---------------- constants
            iota = constp.tile([P, P], BF16, tag="iota")
            nc.gpsimd.iota(iota[:], [[1, P]], channel_multiplier=0,
                           allow_small_or_imprecise_dtypes=True)
            iota_kp = constp.tile([P, P, K], BF16, tag="iota_kp")
            nc.gpsimd.iota(iota_kp[:], [[1, P], [0, K]], channel_multiplier=0,
                           allow_small_or_imprecise_dtypes=True)
            from concourse.masks import make_identity
            ident = constp.tile([P, P], BF16, tag="ident")
            make_identity(nc, ident[:])

            def load_const(name, param, shape, dt):
                t = constp.tile(shape, dt, tag=name, name=name)
                nc.sync.dma_start(out=t[:], in_=param[:, :])
                return t

            srcA_sb = load_const("srcA_sb", srcA16, [P, c.NT * KA * 8], I16)
            srcB_sb = load_const("srcB_sb", srcB16, [P, c.NT * KB * 8], I16)
            dstpos_sb = load_const("dstpos_sb", dstpos, [P, c.NT * K], BF16)
            erbA_sb = load_const("erbA_sb", erbA16, [P, ERR // 16], I16)
            erbB_sb = load_const("erbB_sb", erbB16, [P, ERR // 16], I16)
            mask_sb = load_const("mask_sb", maskAB, [P, 2], F32)

            rhsW = [[constp.tile([P, D2H], BF16,
                                 tag=f"rhsW{l}_{kb}", name=f"rhsW{l}_{kb}")
                     for kb in range(2)] for l in range(2)]
            for l, rt in enumerate([rhs1, rhs2]):
                for kb in range(2):
                    nc.sync.dma_start(out=rhsW[l][kb][:],
                                      in_=rt[kb * P: (kb + 1) * P, :])
            b_bc = [constp.tile([P, c.D], F32, tag=f"bbc{l}", name=f"bbc{l}")
                    for l in range(2)]
            for l, bt in enumerate([b1row, b2row]):
                nc.sync.dma_start(out=b_bc[l][:],
                                  in_=bt[0:1, :].to_broadcast([P, c.D]))

            # Pre-touch consts on compute engines so first uses don't carry
            # extra sync waits on deep pipelines.
            warm = constp.tile([P, 4], F32, tag="warm")
            warmb = warm[:].bitcast(BF16)
            nc.vector.tensor_copy(out=warm[:, 0:1], in_=dstpos_sb[:, 0:1])
            nc.vector.tensor_copy(out=warmb[:, 2:3], in_=iota[:, 0:1])
            nc.vector.tensor_copy(out=warm[:, 1:2], in_=mask_sb[:, 0:1])

            er_tab = [ertabp.tile([P, c.NT, c.H], BF16, tag=f"ertab{l}",
                                  name=f"ertab{l}")
                      for l in range(2)]

            # ---------------- phase A layer 1 (replicated, batched groups)
            def phase_a1():
                ngrp = math.ceil(c.NTA / c.G1)
                for grp in range(ngrp):
                    t0 = grp * c.G1
                    gt = min(c.G1, c.NTA - t0)
                    cols = min(gt * P, c.N - t0 * P)
                    slab = slabp.tile([P, 2, c.G1 * P], BF16, tag="slab")
                    nc.scalar.dma_start(
                        out=slab[:, :, :cols],
                        in_=xT[:, t0 * P: t0 * P + cols]
                        .rearrange("(kb p) n -> p kb n", p=P))
                    pkg = pkgp.tile([P, c.G1, c.ROWF], F32, tag="pkg")
                    pkgb = pkg[:].bitcast(BF16)
                    for j in range(gt):
                        m = min(P, c.N - (t0 + j) * P)
                        ps = psA.tile([P, D2H], F32, tag="psA")
                        for kb in range(2):
                            nc.tensor.matmul(
                                out=ps[:m, :],
                                lhsT=slab[:, kb, j * P: j * P + m],
                                rhs=rhsW[0][kb][:],
                                start=(kb == 0), stop=(kb == 1))
                        if j % 2 == 0:
                            nc.scalar.copy(out=pkgb[:m, j, : c.D],
                                           in_=ps[:m, : c.D])
                        else:
                            nc.vector.tensor_copy(out=pkgb[:m, j, : c.D],
                                                  in_=ps[:m, : c.D])
                        nc.vector.tensor_copy(
                            out=pkg[:m, j, c.D // 2: WC],
                            in_=ps[:m, c.D: D2H])
                    nc.sync.dma_start(
                        out=tab1[t0 * P: (t0 + gt) * P, :WC]
                        .rearrange("(j p) w -> p j w", p=P),
                        in_=pkg[:, :gt, :WC])

            # ---------------- er_tab build for layer 1 (2 gathers + merge)
            def build_er1():
                start = c.ROWF - 64
                eroff = (c.D // 2 + c.H) - start
                half_nt = (c.NT + 1) // 2
                for piece in range(2):
                    tlo = piece * half_nt
                    tn = min(half_nt, c.NT - tlo)
                    ebs = []
                    for half, (r0, r1, idx_sb) in enumerate([
                            (0, c.SPLIT, erbA_sb),
                            (c.SPLIT, NPAD, erbB_sb)]):
                        eb = ebp.tile([P, half_nt, 64], F32,
                                      tag=f"eb{half}", name=f"eb{half}")
                        nc.gpsimd.dma_gather(
                            out_ap=eb[:, :tn, :],
                            in_ap=tab1[r0:r1, start: start + 64],
                            idxs_ap=idx_sb[:, tlo * 8: (tlo + tn) * 8],
                            num_idxs=tn * P, num_idxs_reg=tn * P,
                            elem_size=64, elem_step=c.ROWF,
                            single_packet=False, queue_num=half)
                        ebs.append(eb)
                    tmp = smallp.tile([P, half_nt, c.H], F32, tag="ermerge")
                    nc.vector.tensor_scalar(
                        out=tmp[:, :tn], in0=ebs[0][:, :tn, eroff:eroff + c.H],
                        scalar1=mask_sb[:, 0:1], scalar2=None,
                        op0=mybir.AluOpType.mult)
                    nc.vector.scalar_tensor_tensor(
                        out=er_tab[0][:, tlo: tlo + tn],
                        in0=ebs[1][:, :tn, eroff:eroff + c.H],
                        scalar=mask_sb[:, 1:2], in1=tmp[:, :tn],
                        op0=mybir.AluOpType.mult, op1=mybir.AluOpType.add)

            # ---------------- phase B (layer = 0 or 1)
            def phase_b(layer):
                tabA0 = tab1 if layer == 0 else tab2
                ngrp = math.ceil(c.NT / c.GB)
                for grp in range(ngrp):
                    t0 = grp * c.GB
                    gt = min(c.GB, c.NT - t0)
                    if layer == 0:
                        pkg2 = pkgp.tile([P, c.GB, c.ROWF], F32, tag="pkg2")
                        pkg2b = pkg2[:].bitcast(BF16)
                    else:
                        o2g = outp.tile([P, c.GB, c.D], F32, tag="o2g")
                    for j in range(gt):
                        t = t0 + j
                        g = gathp.tile([P, K, c.ROWF], F32, tag="gath")
                        nc.gpsimd.dma_gather(
                            out_ap=g[:, 0:KA, :], in_ap=tabA0[0: c.SPLIT, :],
                            idxs_ap=srcA_sb[:, t * KA * 8:(t + 1) * KA * 8],
                            num_idxs=KA * P, num_idxs_reg=KA * P,
                            elem_size=c.ROWF, single_packet=KA * P <= 1024)
                        nc.gpsimd.dma_gather(
                            out_ap=g[:, KA:K, :],
                            in_ap=tabA0[c.SPLIT: c.N, :],
                            idxs_ap=srcB_sb[:, t * KB * 8:(t + 1) * KB * 8],
                            num_idxs=KB * P, num_idxs_reg=KB * P,
                            elem_size=c.ROWF, single_packet=KB * P <= 1024,
                            queue_num=1)
                        gb = g[:].bitcast(BF16)

                        # transposed one-hots for this tile (host data)
                        M_all = onehotp.tile([P, K, P], FP8, tag="M")
                        nc.scalar.dma_start(
                            out=M_all[:],
                            in_=Mhot[:, t * K * P: (t + 1) * K * P]
                            .rearrange("p (k q) -> p k q", k=K))

                        # er per slot: K tiny matmuls M_all[:, ck, :]^T @ er_tab
                        erp = psE.tile([P, K * c.H], F32, tag="psE")
                        for ck in range(K):
                            nc.tensor.matmul(
                                out=erp[:, ck * c.H:(ck + 1) * c.H],
                                lhsT=M_all[:, ck, :],
                                rhs=er_tab[layer][:, t, :],
                                start=True, stop=True)

                        # e = leakyrelu(el + er); exp
                        ea = smallp.tile([P, K, c.H], F32, tag="eadd")
                        nc.vector.tensor_tensor(
                            out=ea[:], in0=g[:, :, c.D // 2: c.D // 2 + c.H],
                            in1=erp[:].rearrange("p (k h) -> p k h", h=c.H),
                            op=mybir.AluOpType.add)
                        lr = smallp.tile([P, K, c.H], F32, tag="lrout")
                        nc.vector.scalar_tensor_tensor(
                            out=lr[:], in0=ea[:], scalar=c.NEG, in1=ea[:],
                            op0=mybir.AluOpType.mult, op1=mybir.AluOpType.max)
                        rm = rhsmp.tile([P, K, DH], BF16, tag="rhsm")
                        nc.scalar.activation(
                            out=rm[:, :, c.D: DH], in_=lr[:],
                            func=mybir.ActivationFunctionType.Exp)
                        expb = rm[:, :, c.D: DH].unsqueeze(2).to_broadcast(
                            [P, K, c.HD, c.H])
                        feat4 = gb[:, :, : c.D].rearrange(
                            "p k (hd h) -> p k hd h", h=c.H)
                        out4 = rm[:, :, : c.D].rearrange(
                            "p k (hd h) -> p k hd h", h=c.H)
                        nc.vector.tensor_tensor(out=out4, in0=feat4, in1=expb,
                                                op=mybir.AluOpType.mult)

                        # segment sum via one-hot matmuls; all K S-chunks
                        # generated in one 2x-eligible DVE op (layout [P,P,K])
                        S_allT = selp.tile([P, P, K], BF16, tag="S_allT")
                        nc.vector.tensor_tensor(
                            out=S_allT[:],
                            in0=iota_kp[:],
                            in1=dstpos_sb[:, t * K:(t + 1) * K]
                            .unsqueeze(1).to_broadcast([P, P, K]),
                            op=mybir.AluOpType.is_equal)
                        ps = psB.tile([P, DH], F32, tag="psB")
                        for ck in range(K):
                            nc.tensor.matmul(out=ps[:], lhsT=S_allT[:, :, ck],
                                             rhs=rm[:, ck, :],
                                             start=(ck == 0), stop=(ck == K - 1))
                        den = smallp.tile([P, c.H], F32, tag="den")
                        nc.vector.tensor_scalar_max(den[:], ps[:, c.D: DH], 1e-30)
                        rcp = smallp.tile([P, c.H], F32, tag="rcp")
                        nc.vector.reciprocal(rcp[:], den[:])
                        o1 = outp.tile([P, c.D], F32, tag="o1")
                        rcpb = rcp[:].unsqueeze(1).to_broadcast([P, c.HD, c.H])
                        ps4 = ps[:, : c.D].rearrange("p (hd h) -> p hd h", h=c.H)
                        o14 = o1[:].rearrange("p (hd h) -> p hd h", h=c.H)
                        nc.vector.tensor_tensor(out=o14, in0=ps4, in1=rcpb,
                                                op=mybir.AluOpType.mult)
                        nc.vector.tensor_tensor(out=o1[:], in0=o1[:],
                                                in1=b_bc[layer][:],
                                                op=mybir.AluOpType.add)
                        if layer == 0:
                            # h for this tile -> layer-2 phase A (sharded)
                            hb = outp.tile([P, c.D], BF16, tag="hb")
                            nc.vector.tensor_scalar_max(hb[:], o1[:], 0.0)
                            ps2 = psA.tile([P, D2H], F32, tag="psA")
                            for kb in range(2):
                                pst = psT.tile([P, P], BF16, tag="psT")
                                nc.tensor.transpose(
                                    out=pst[:],
                                    in_=hb[:, kb * P: (kb + 1) * P],
                                    identity=ident[:])
                                lt2 = lt2p.tile([P, P], BF16, tag="lt2")
                                nc.scalar.copy(out=lt2[:], in_=pst[:])
                                nc.tensor.matmul(
                                    out=ps2[:], lhsT=lt2[:],
                                    rhs=rhsW[1][kb][:],
                                    start=(kb == 0), stop=(kb == 1))
                            nc.scalar.copy(out=pkg2b[:, j, : c.D],
                                           in_=ps2[:, : c.D])
                            nc.vector.tensor_copy(
                                out=pkg2[:, j, c.D // 2: c.D // 2 + c.H],
                                in_=ps2[:, c.D: c.D + c.H])
                            nc.vector.tensor_copy(
                                out=er_tab[1][:, t, :],
                                in_=ps2[:, c.D + c.H: D2H])
                        else:
                            nc.vector.tensor_scalar_max(o2g[:, j, :], o1[:], 0.0)
                    # group epilogue
                    if layer == 0:
                        rows = min(gt * P, c.NPC - t0 * P)
                        full = rows // P
                        WEL = c.D // 2 + c.H
                        if full:
                            nc.sync.dma_start(
                                out=tab2loc[t0 * P: t0 * P + full * P, :WEL]
                                .rearrange("(j p) w -> p j w", p=P),
                                in_=pkg2[:, :full, :WEL])
                        tail = rows - full * P
                        if tail:
                            nc.sync.dma_start(
                                out=tab2loc[t0 * P + full * P:
                                            t0 * P + full * P + tail, :WEL],
                                in_=pkg2[:tail, full, :WEL])
                    else:
                        nc.sync.dma_start(
                            out=out_ext[t0 * P: (t0 + gt) * P, :]
                            .rearrange("(j p) d -> p j d", p=P),
                            in_=o2g[:, :gt, :])

            for _rep in range(reps):
                phase_a1()
                build_er1()
                phase_b(0)
                nc.gpsimd.collective_compute(
                    "AllGather",
                    mybir.AluOpType.bypass,
                    replica_groups=[list(range(c.cores))],
                    ins=[tab2loc[:]],
                    outs=[tab2[:]],
                )
                phase_b(1)

    nc.compile()
    return nc


# ---------------------------------------------------------------- reference

def ref_np(inputs, cfg):
    c = cfg
    x = np.asarray(inputs["data"], np.float64)
    src = np.asarray(inputs["src"]).astype(np.int64)
    dst = np.asarray(inputs["dst"]).astype(np.int64)

    def layer(x, W, al, ar, b):
        N = x.shape[0]
        feat = (x @ np.asarray(W, np.float64)).reshape(N, c.H, c.HD)
        el = np.einsum("nhd,hd->nh", feat, np.asarray(al, np.float64))
        er = np.einsum("nhd,hd->nh", feat, np.asarray(ar, np.float64))
        e = el[src] + er[dst]
        e = np.where(e > 0, e, c.NEG * e)
        m = np.full((N, c.H), -np.inf)
        np.maximum.at(m, dst, e)
        a = np.exp(e - m[dst])
        den = np.zeros((N, c.H))
        np.add.at(den, dst, a)
        alpha = a / den[dst]
        msg = feat[src] * alpha[:, :, None]
        out = np.zeros((N, c.H, c.HD))
        np.add.at(out, dst, msg)
        out = out + np.asarray(b, np.float64).reshape(1, c.H, c.HD)
        return np.maximum(out, 0).reshape(N, c.D)

    h = layer(x, inputs["W1"], inputs["al1"], inputs["ar1"], inputs["b1"])
    h = layer(h, inputs["W2"], inputs["al2"], inputs["ar2"], inputs["b2"])
    return h


# ---------------------------------------------------------------- entry point

_BUILD_CACHE = {}


def kernel(**inputs) -> np.ndarray:
    """Full-input GAT kernel: shards internally across 8 NeuronCores."""
    from concourse.bass_utils import run_bass_kernel_spmd

    cfg = make_cfg(N=50000, E=800000, D=256, H=8, cores=8)
    in_maps, meta = prep_all(inputs, cfg)
    key = (meta.KA, meta.KB)
    if key not in _BUILD_CACHE:
        _BUILD_CACHE[key] = build_nc(cfg, meta.KA, meta.KB)
    nc = _BUILD_CACHE[key]
    res = run_bass_kernel_spmd(nc, in_maps, list(range(cfg.cores)))
    results = [{"out": res.results[ci]["out"]} for ci in range(cfg.cores)]
    out = finalize(results, cfg, meta)
    return np.ascontiguousarray(out.astype(np.float32))


# revision 5
# speedup vs baseline: 1.0089x; 1.0089x over previous
"""2-layer GAT (DGL GATConv style) on 8 trn2 NeuronCores via Bass/Tile. v2

Design:
  - Edges dst-sorted on host; cores own contiguous equal node ranges
    [c*NPC, (c+1)*NPC) and all edges whose dst falls in range.
  - Layer 1 phase A (replicated): packed table row (f32 container)
    [feat(D bf16) | el(H f32) | er(H f32) | pad] = x @ [W | Wal | War],
    written to tab1 in groups of 8 tiles (one slab load + one table write
    per group). Feature cols h-innermost (packed col j <-> head j%H).
  - er for own nodes lives in SBUF (er_tab[d, t, h], d = dst position in
    tile t). Layer 1 builds it with two small gathers (tabA/tabB halves,
    host-clipped indices) + mask merge; layer 2 captures it for free from
    phase-A PSUM (the layer-2 transform is sharded, tiles align).
  - Phase B (sharded): per dst-tile, gather src rows (768 B each) with
    dma_gather. S one-hots (slot->dst, for segment-sum matmuls) are
    generated in ONE 2x-rate DVE op per tile (layout [P, P, K] keeps every
    operand innermost-packed); M one-hots (dst->slot transposes, for the
    er broadcast) are host-built fp8 data streamed from DRAM (0/1 is exact
    in fp8; half the bytes of bf16). er per slot = K tiny M^T @ er_tab
    matmuls instead of a 256 B/edge gather. exp(leakyrelu(el+er)) ->
    weighted messages -> segment sum via S matmuls into PSUM [num | den];
    normalize, bias, relu.
  - Layer 2 phase A is fused into phase B of layer 1: each B0 tile's
    output hb is PE-transposed and immediately transformed (2 matmuls) to
    its packed table row; rows go to tab2loc [NPC, ROWF]. After B0 an
    AllGather concatenates the 8 slices into the full tab2 - no hT
    roundtrip and no replicated layer-2 transform.
"""

import math
import sys
from types import SimpleNamespace

import numpy as np

sys.path.insert(0, "/opt/trn_rl_repo")

from concourse import bacc, bass, mybir, tile  # noqa: E402

F32 = mybir.dt.float32
BF16 = mybir.dt.bfloat16
FP8 = mybir.dt.float8e4
I32 = mybir.dt.int32
I16 = mybir.dt.int16

P = 128


def make_cfg(N=50000, E=800000, D=256, H=8, cores=8, split=None):
    HD = D // H
    NPC = N // cores
    NT = math.ceil(NPC / P)          # dst tiles per core
    NTA = math.ceil(N / P)           # phase-A node tiles (layer 1, flat)
    ROWU = ((D + 4 * H + 127) // 128) * 128   # packed row, bf16 units (256B mult)
    if split is None:
        split = NPC * min(cores, 32767 // NPC)
        split = min(split, N)
    assert split % NPC == 0 and split <= 32767 + 1 and N - split <= 32767 + 1
    return SimpleNamespace(
        N=N, E=E, D=D, H=H, HD=HD, cores=cores, NPC=NPC, NT=NT, NTA=NTA,
        ROWU=ROWU, ROWF=ROWU // 2, SPLIT=split,
        G1=8, GB=7,
        NEG=0.2,
    )


# ---------------------------------------------------------------- host prep

def perm_h_inner(D, H):
    """perm[j] = original feature index stored at packed col j (h-innermost)."""
    HD = D // H
    j = np.arange(D)
    return (j % H) * HD + j // H


def attn_cols(W, a, H):
    D = W.shape[0]
    HD = W.shape[1] // H
    return np.stack(
        [W[:, h * HD:(h + 1) * HD] @ a[h] for h in range(H)], axis=1
    )


def wrap16(flat, reps=8):
    """[num] -> [16*reps, num//16] int16 wrapped layout (idx i at [i%16, i//16]),
    replicated across the 8 gpsimd cores."""
    num = len(flat)
    assert num % 16 == 0
    a = np.zeros((16, num // 16), dtype=np.int16)
    a[np.arange(num) % 16, np.arange(num) // 16] = flat
    return np.tile(a, (reps, 1))


def prep_edges(src, dst, cfg):
    c = cfg
    order = np.argsort(dst, kind="stable")
    src_s = src[order].astype(np.int64)
    dst_s = dst[order].astype(np.int64)

    core = dst_s // c.NPC
    loc = dst_s - core * c.NPC
    lt = loc // P
    pos = loc - lt * P
    islow = src_s < c.SPLIT

    # group = (core, tile, islow) ; edges of a group get consecutive slots
    g = (core * c.NT + lt) * 2 + (1 - islow)   # low first
    order2 = np.argsort(g, kind="stable")
    src_s, dst_s, core, lt, pos, islow, g = (
        a[order2] for a in (src_s, dst_s, core, lt, pos, islow, g))

    uniq, starts = np.unique(g, return_index=True)
    start_of = np.zeros(c.cores * c.NT * 2, dtype=np.int64)
    start_of[uniq] = starts
    rank = np.arange(len(dst_s)) - start_of[g]

    counts = np.zeros(c.cores * c.NT * 2, dtype=np.int64)
    np.add.at(counts, g, 1)
    KA = int(math.ceil(counts[0::2].max() / P))
    KB = int(math.ceil(counts[1::2].max() / P)) if counts[1::2].max() > 0 else 0
    K = KA + KB

    # slot within tile: low edges fill chunks [0,KA), high fill [KA,K)
    chunk = rank // P + np.where(islow, 0, KA)
    part = rank % P

    srcA = np.zeros((c.cores, c.NT * KA * P), dtype=np.int64)
    srcB = np.zeros((c.cores, c.NT * KB * P), dtype=np.int64) if KB else None
    dstpos = np.full((c.cores, P, c.NT * K), 255.0, dtype=np.float32)
    dstposT = np.full((c.cores, c.NT * K * P), 255, dtype=np.int16)

    iA = lt * (KA * P) + (chunk * P + part)            # valid where islow
    iB = lt * (KB * P) + ((chunk - KA) * P + part) if KB else None
    low = islow
    srcA[core[low], iA[low]] = src_s[low]
    if KB:
        hi = ~islow
        srcB[core[hi], iB[hi]] = src_s[hi] - c.SPLIT
    dstpos[core, part, lt * K + chunk] = pos
    dstposT[core, (lt * K + chunk) * P + part] = pos

    # host-built transposed one-hots M[d, (t, ck, p)] = (dstposT == d).
    # fp8e4m3 is exact for 0/1 and halves the stream bytes.
    import ml_dtypes
    Mhot = (dstposT[:, None, :] == np.arange(P, dtype=np.int16)[None, :, None]
            ).astype(ml_dtypes.float8_e4m3fn)

    srcA16 = np.stack([wrap16(srcA[ci]) for ci in range(c.cores)])
    srcB16 = (np.stack([wrap16(srcB[ci]) for ci in range(c.cores)])
              if KB else np.zeros((c.cores, P, 0), np.int16))
    return srcA16, srcB16, dstpos, Mhot, KA, KB


def prep_all(inputs, cfg):
    c = cfg
    perm = perm_h_inner(c.D, c.H)
    x = np.asarray(inputs["data"], np.float32)
    src = np.asarray(inputs["src"]).astype(np.int64)
    dst = np.asarray(inputs["dst"]).astype(np.int64)

    def rhs_for(W, al, ar, permute_rows):
        W = np.asarray(W, np.float64)
        Wal = attn_cols(W, np.asarray(al, np.float64), c.H)
        War = attn_cols(W, np.asarray(ar, np.float64), c.H)
        Wp = W[:, perm]
        if permute_rows:
            Wp, Wal, War = Wp[perm], Wal[perm], War[perm]
        return to_bf16(np.concatenate([Wp, Wal, War], axis=1))

    rhs1 = rhs_for(inputs["W1"], inputs["al1"], inputs["ar1"], False)
    rhs2 = rhs_for(inputs["W2"], inputs["al2"], inputs["ar2"], True)
    b1 = np.asarray(inputs["b1"], np.float32)[perm].reshape(1, c.D)
    b2 = np.asarray(inputs["b2"], np.float32)[perm].reshape(1, c.D)
    xT = to_bf16(x.T.copy())

    srcA16, srcB16, dstpos, Mhot, KA, KB = prep_edges(src, dst, c)

    # er build indices (layer 1): row i (i in [0, NT*128)) <- own node NPC*ci+i
    erbA, erbB, masks = [], [], []
    for ci in range(c.cores):
        base = ci * c.NPC
        rows = np.arange(c.NT * P)
        ra = np.where(base < c.SPLIT, base + rows, 0)
        ra = np.clip(ra, 0, c.SPLIT - 1)
        rb = np.where(base >= c.SPLIT, base - c.SPLIT + rows, 0)
        rb = np.clip(rb, 0, max(c.N - c.SPLIT - 1, 0))
        erbA.append(wrap16(ra))
        erbB.append(wrap16(rb))
        mA = 1.0 if base < c.SPLIT else 0.0
        m = np.zeros((P, 2), np.float32)
        m[:, 0] = mA
        m[:, 1] = 1.0 - mA
        masks.append(m)

    meta = SimpleNamespace(perm=perm, KA=KA, KB=KB, K=KA + KB)
    in_maps = []
    for ci in range(c.cores):
        in_maps.append({
            "xT": xT, "rhs1": rhs1, "rhs2": rhs2, "b1row": b1, "b2row": b2,
            "srcA16": srcA16[ci], "srcB16": srcB16[ci],
            "dstpos": to_bf16(dstpos[ci]),
            "Mhot": Mhot[ci],
            "erbA16": erbA[ci], "erbB16": erbB[ci],
            "maskAB": masks[ci],
        })
    return in_maps, meta


def to_bf16(a):
    import ml_dtypes
    return np.asarray(a).astype(ml_dtypes.bfloat16)


def finalize(results, cfg, meta):
    c = cfg
    parts = [results[ci]["out"][: c.NPC] for ci in range(c.cores)]
    out_p = np.concatenate(parts, axis=0)
    out = np.empty_like(out_p)
    out[:, meta.perm] = out_p
    return out


# ---------------------------------------------------------------- kernel

def build_nc(cfg, KA, KB, debug=False, reps=1):
    c = cfg
    K = KA + KB
    ERR = c.NT * P                   # er rows per core (padded)
    NPAD = c.NTA * P                 # padded table rows (layer 1)
    DH = c.D + c.H                   # seg-matmul rhs cols (msg + den)
    D2H = c.D + 2 * c.H              # phase-A rhs cols
    WC = c.D // 2 + 2 * c.H          # written f32 cols of a packed row

    nc = bacc.Bacc("TRN2", target_bir_lowering=False, debug=debug,
                   num_devices=c.cores, num_swdge_queues=2)

    xT = nc.declare_dram_parameter("xT", [c.D, c.N], BF16, isOutput=False)
    rhs1 = nc.declare_dram_parameter("rhs1", [c.D, D2H], BF16, isOutput=False)
    rhs2 = nc.declare_dram_parameter("rhs2", [c.D, D2H], BF16, isOutput=False)
    b1row = nc.declare_dram_parameter("b1row", [1, c.D], F32, isOutput=False)
    b2row = nc.declare_dram_parameter("b2row", [1, c.D], F32, isOutput=False)
    srcA16 = nc.declare_dram_parameter("srcA16", [P, c.NT * KA * 8], I16, isOutput=False)
    srcB16 = nc.declare_dram_parameter("srcB16", [P, c.NT * KB * 8], I16, isOutput=False)
    dstpos = nc.declare_dram_parameter("dstpos", [P, c.NT * K], BF16, isOutput=False)
    Mhot = nc.declare_dram_parameter("Mhot", [P, c.NT * K * P], FP8, isOutput=False)
    erbA16 = nc.declare_dram_parameter("erbA16", [P, ERR // 16], I16, isOutput=False)
    erbB16 = nc.declare_dram_parameter("erbB16", [P, ERR // 16], I16, isOutput=False)
    maskAB = nc.declare_dram_parameter("maskAB", [P, 2], F32, isOutput=False)
    out_ext = nc.declare_dram_parameter("out", [c.NT * P, c.D], F32, isOutput=True)

    tab1 = nc.dram_tensor("tab1", [NPAD, c.ROWF], F32)
    tab2loc = nc.dram_tensor("tab2loc", [c.NPC, c.ROWF], F32)
    tab2 = nc.dram_tensor("tab2", [c.N, c.ROWF], F32,
                          addr_space="Shared" if c.cores > 4 else "Local")

    with tile.TileContext(nc) as tc:
        with (
            tc.tile_pool(name="const", bufs=1) as constp,
            tc.tile_pool(name="slab", bufs=3) as slabp,
            tc.tile_pool(name="pkg", bufs=2) as pkgp,
            tc.tile_pool(name="eb", bufs=1) as ebp,
            tc.tile_pool(name="ertab", bufs=1) as ertabp,
            tc.tile_pool(name="gath", bufs=4) as gathp,
            tc.tile_pool(name="onehot", bufs=3) as onehotp,
            tc.tile_pool(name="sel", bufs=4) as selp,
            tc.tile_pool(name="rhsm", bufs=2) as rhsmp,
            tc.tile_pool(name="small", bufs=3) as smallp,
            tc.tile_pool(name="lt2", bufs=3) as lt2p,
            tc.tile_pool(name="outp", bufs=2) as outp,
            tc.tile_pool(name="psA", bufs=4, space="PSUM") as psA,
            tc.tile_pool(name="psB", bufs=2, space="PSUM") as psB,
            tc.tile_pool(name="psE", bufs=1, space="PSUM") as psE,
            tc.tile_pool(name="psT", bufs=1, space="PSUM") as psT,
        ):
            # ---------------- constants
            iota = constp.tile([P, P], BF16, tag="iota")
            nc.gpsimd.iota(iota[:], [[1, P]], channel_multiplier=0,
                           allow_small_or_imprecise_dtypes=True)
            iota_kp = constp.tile([P, P, K], BF16, tag="iota_kp")
            nc.gpsimd.iota(iota_kp[:], [[1, P], [0, K]], channel_multiplier=0,
                           allow_small_or_imprecise_dtypes=True)
            from concourse.masks import make_identity
            ident = constp.tile([P, P], BF16, tag="ident")
            make_identity(nc, ident[:])

            def load_const(name, param, shape, dt):
                t = constp.tile(shape, dt, tag=name, name=name)
                nc.sync.dma_start(out=t[:], in_=param[:, :])
                return t

            srcA_sb = load_const("srcA_sb", srcA16, [P, c.NT * KA * 8], I16)
            srcB_sb = load_const("srcB_sb", srcB16, [P, c.NT * KB * 8], I16)
            dstpos_sb = load_const("dstpos_sb", dstpos, [P, c.NT * K], BF16)
            erbA_sb = load_const("erbA_sb", erbA16, [P, ERR // 16], I16)
            erbB_sb = load_const("erbB_sb", erbB16, [P, ERR // 16], I16)
            mask_sb = load_const("mask_sb", maskAB, [P, 2], F32)

            rhsW = [[constp.tile([P, D2H], BF16,
                                 tag=f"rhsW{l}_{kb}", name=f"rhsW{l}_{kb}")
                     for kb in range(2)] for l in range(2)]
            for l, rt in enumerate([rhs1, rhs2]):
                for kb in range(2):
                    nc.sync.dma_start(out=rhsW[l][kb][:],
                                      in_=rt[kb * P: (kb + 1) * P, :])
            b_bc = [constp.tile([P, c.D], F32, tag=f"bbc{l}", name=f"bbc{l}")
                    for l in range(2)]
            for l, bt in enumerate([b1row, b2row]):
                nc.sync.dma_start(out=b_bc[l][:],
                                  in_=bt[0:1, :].to_broadcast([P, c.D]))

            # Pre-touch consts on compute engines so first uses don't carry
            # extra sync waits on deep pipelines.
            warm = constp.tile([P, 4], F32, tag="warm")
            warmb = warm[:].bitcast(BF16)
            nc.vector.tensor_copy(out=warm[:, 0:1], in_=dstpos_sb[:, 0:1])
            nc.vector.tensor_copy(out=warmb[:, 2:3], in_=iota[:, 0:1])
            nc.vector.tensor_copy(out=warm[:, 1:2], in_=mask_sb[:, 0:1])

            er_tab = [ertabp.tile([P, c.NT, c.H], BF16, tag=f"ertab{l}",
                                  name=f"ertab{l}")
                      for l in range(2)]

            # ---------------- phase A layer 1 (replicated, batched groups)
            def phase_a1():
                ngrp = math.ceil(c.NTA / c.G1)
                for grp in range(ngrp):
                    t0 = grp * c.G1
                    gt = min(c.G1, c.NTA - t0)
                    cols = min(gt * P, c.N - t0 * P)
                    slab = slabp.tile([P, 2, c.G1 * P], BF16, tag="slab")
                    nc.scalar.dma_start(
                        out=slab[:, :, :cols],
                        in_=xT[:, t0 * P: t0 * P + cols]
                        .rearrange("(kb p) n -> p kb n", p=P))
                    pkg = pkgp.tile([P, c.G1, c.ROWF], F32, tag="pkg")
                    pkgb = pkg[:].bitcast(BF16)
                    for j in range(gt):
                        m = min(P, c.N - (t0 + j) * P)
                        ps = psA.tile([P, D2H], F32, tag="psA")
                        for kb in range(2):
                            nc.tensor.matmul(
                                out=ps[:m, :],
                                lhsT=slab[:, kb, j * P: j * P + m],
                                rhs=rhsW[0][kb][:],
                                start=(kb == 0), stop=(kb == 1))
                        if j % 2 == 0:
                            nc.scalar.copy(out=pkgb[:m, j, : c.D],
                                           in_=ps[:m, : c.D])
                            nc.vector.tensor_copy(
                                out=pkg[:m, j, c.D // 2: WC],
                                in_=ps[:m, c.D: D2H])
                        else:
                            nc.vector.tensor_copy(out=pkgb[:m, j, : c.D],
                                                  in_=ps[:m, : c.D])
                            nc.scalar.copy(
                                out=pkg[:m, j, c.D // 2: WC],
                                in_=ps[:m, c.D: D2H])
                    nc.sync.dma_start(
                        out=tab1[t0 * P: (t0 + gt) * P, :WC]
                        .rearrange("(j p) w -> p j w", p=P),
                        in_=pkg[:, :gt, :WC])

            # ---------------- er_tab build for layer 1 (2 gathers + merge)
            def build_er1():
                start = c.ROWF - 64
                eroff = (c.D // 2 + c.H) - start
                half_nt = (c.NT + 1) // 2
                for piece in range(2):
                    tlo = piece * half_nt
                    tn = min(half_nt, c.NT - tlo)
                    ebs = []
                    for half, (r0, r1, idx_sb) in enumerate([
                            (0, c.SPLIT, erbA_sb),
                            (c.SPLIT, NPAD, erbB_sb)]):
                        eb = ebp.tile([P, half_nt, 64], F32,
                                      tag=f"eb{half}", name=f"eb{half}")
                        nc.gpsimd.dma_gather(
                            out_ap=eb[:, :tn, :],
                            in_ap=tab1[r0:r1, start: start + 64],
                            idxs_ap=idx_sb[:, tlo * 8: (tlo + tn) * 8],
                            num_idxs=tn * P, num_idxs_reg=tn * P,
                            elem_size=64, elem_step=c.ROWF,
                            single_packet=False, queue_num=half)
                        ebs.append(eb)
                    tmp = smallp.tile([P, half_nt, c.H], F32, tag="ermerge")
                    nc.vector.tensor_scalar(
                        out=tmp[:, :tn], in0=ebs[0][:, :tn, eroff:eroff + c.H],
                        scalar1=mask_sb[:, 0:1], scalar2=None,
                        op0=mybir.AluOpType.mult)
                    nc.vector.scalar_tensor_tensor(
                        out=er_tab[0][:, tlo: tlo + tn],
                        in0=ebs[1][:, :tn, eroff:eroff + c.H],
                        scalar=mask_sb[:, 1:2], in1=tmp[:, :tn],
                        op0=mybir.AluOpType.mult, op1=mybir.AluOpType.add)

            # ---------------- phase B (layer = 0 or 1)
            def phase_b(layer):
                tabA0 = tab1 if layer == 0 else tab2
                ngrp = math.ceil(c.NT / c.GB)
                for grp in range(ngrp):
                    t0 = grp * c.GB
                    gt = min(c.GB, c.NT - t0)
                    if layer == 0:
                        pkg2 = pkgp.tile([P, c.GB, c.ROWF], F32, tag="pkg2")
                        pkg2b = pkg2[:].bitcast(BF16)
                    else:
                        o2g = outp.tile([P, c.GB, c.D], F32, tag="o2g")
                    for j in range(gt):
                        t = t0 + j
                        g = gathp.tile([P, K, c.ROWF], F32, tag="gath")
                        nc.gpsimd.dma_gather(
                            out_ap=g[:, 0:KA, :], in_ap=tabA0[0: c.SPLIT, :],
                            idxs_ap=srcA_sb[:, t * KA * 8:(t + 1) * KA * 8],
                            num_idxs=KA * P, num_idxs_reg=KA * P,
                            elem_size=c.ROWF, single_packet=KA * P <= 1024)
                        nc.gpsimd.dma_gather(
                            out_ap=g[:, KA:K, :],
                            in_ap=tabA0[c.SPLIT: c.N, :],
                            idxs_ap=srcB_sb[:, t * KB * 8:(t + 1) * KB * 8],
                            num_idxs=KB * P, num_idxs_reg=KB * P,
                            elem_size=c.ROWF, single_packet=KB * P <= 1024,
                            queue_num=1)
                        gb = g[:].bitcast(BF16)

                        # transposed one-hots for this tile (host data)
                        M_all = onehotp.tile([P, K, P], FP8, tag="M")
                        nc.scalar.dma_start(
                            out=M_all[:],
                            in_=Mhot[:, t * K * P: (t + 1) * K * P]
                            .rearrange("p (k q) -> p k q", k=K))

                        # er per slot: K tiny matmuls M_all[:, ck, :]^T @ er_tab
                        erp = psE.tile([P, K * c.H], F32, tag="psE")
                        for ck in range(K):
                            nc.tensor.matmul(
                                out=erp[:, ck * c.H:(ck + 1) * c.H],
                                lhsT=M_all[:, ck, :],
                                rhs=er_tab[layer][:, t, :],
                                start=True, stop=True)

                        # e = leakyrelu(el + er); exp
                        ea = smallp.tile([P, K, c.H], F32, tag="eadd")
                        nc.vector.tensor_tensor(
                            out=ea[:], in0=g[:, :, c.D // 2: c.D // 2 + c.H],
                            in1=erp[:].rearrange("p (k h) -> p k h", h=c.H),
                            op=mybir.AluOpType.add)
                        lr = smallp.tile([P, K, c.H], F32, tag="lrout")
                        nc.vector.scalar_tensor_tensor(
                            out=lr[:], in0=ea[:], scalar=c.NEG, in1=ea[:],
                            op0=mybir.AluOpType.mult, op1=mybir.AluOpType.max)
                        rm = rhsmp.tile([P, K, DH], BF16, tag="rhsm")
                        nc.scalar.activation(
                            out=rm[:, :, c.D: DH], in_=lr[:],
                            func=mybir.ActivationFunctionType.Exp)
                        expb = rm[:, :, c.D: DH].unsqueeze(2).to_broadcast(
                            [P, K, c.HD, c.H])
                        feat4 = gb[:, :, : c.D].rearrange(
                            "p k (hd h) -> p k hd h", h=c.H)
                        out4 = rm[:, :, : c.D].rearrange(
                            "p k (hd h) -> p k hd h", h=c.H)
                        nc.vector.tensor_tensor(out=out4, in0=feat4, in1=expb,
                                                op=mybir.AluOpType.mult)

                        # segment sum via one-hot matmuls; all K S-chunks
                        # generated in one 2x-eligible DVE op (layout [P,P,K])
                        S_allT = selp.tile([P, P, K], BF16, tag="S_allT")
                        nc.vector.tensor_tensor(
                            out=S_allT[:],
                            in0=iota_kp[:],
                            in1=dstpos_sb[:, t * K:(t + 1) * K]
                            .unsqueeze(1).to_broadcast([P, P, K]),
                            op=mybir.AluOpType.is_equal)
                        ps = psB.tile([P, DH], F32, tag="psB")
                        for ck in range(K):
                            nc.tensor.matmul(out=ps[:], lhsT=S_allT[:, :, ck],
                                             rhs=rm[:, ck, :],
                                             start=(ck == 0), stop=(ck == K - 1))
                        den = smallp.tile([P, c.H], F32, tag="den")
                        nc.vector.tensor_scalar_max(den[:], ps[:, c.D: DH], 1e-30)
                        rcp = smallp.tile([P, c.H], F32, tag="rcp")
                        nc.vector.reciprocal(rcp[:], den[:])
                        o1 = outp.tile([P, c.D], F32, tag="o1")
                        rcpb = rcp[:].unsqueeze(1).to_broadcast([P, c.HD, c.H])
                        ps4 = ps[:, : c.D].rearrange("p (hd h) -> p hd h", h=c.H)
                        o14 = o1[:].rearrange("p (hd h) -> p hd h", h=c.H)
                        nc.vector.tensor_tensor(out=o14, in0=ps4, in1=rcpb,
                                                op=mybir.AluOpType.mult)
                        nc.vector.tensor_tensor(out=o1[:], in0=o1[:],
                                                in1=b_bc[layer][:],
                                                op=mybir.AluOpType.add)
                        if layer == 0:
                            # h for this tile -> layer-2 phase A (sharded)
                            hb = outp.tile([P, c.D], BF16, tag="hb")
                            nc.vector.tensor_scalar_max(hb[:], o1[:], 0.0)
                            ps2 = psA.tile([P, D2H], F32, tag="psA")
                            for kb in range(2):
                                pst = psT.tile([P, P], BF16, tag="psT")
                                nc.tensor.transpose(
                                    out=pst[:],
                                    in_=hb[:, kb * P: (kb + 1) * P],
                                    identity=ident[:])
                                lt2 = lt2p.tile([P, P], BF16, tag="lt2")
                                nc.scalar.copy(out=lt2[:], in_=pst[:])
                                nc.tensor.matmul(
                                    out=ps2[:], lhsT=lt2[:],
                                    rhs=rhsW[1][kb][:],
                                    start=(kb == 0), stop=(kb == 1))
                            nc.scalar.copy(out=pkg2b[:, j, : c.D],
                                           in_=ps2[:, : c.D])
                            nc.scalar.copy(
                                out=pkg2[:, j, c.D // 2: c.D // 2 + c.H],
                                in_=ps2[:, c.D: c.D + c.H])
                            nc.scalar.copy(
                                out=er_tab[1][:, t, :],
                                in_=ps2[:, c.D + c.H: D2H])
                        else:
                            nc.vector.tensor_scalar_max(o2g[:, j, :], o1[:], 0.0)
                    # group epilogue
                    if layer == 0:
                        rows = min(gt * P, c.NPC - t0 * P)
                        full = rows // P
                        WEL = c.D // 2 + c.H
                        if full:
                            nc.sync.dma_start(
                                out=tab2loc[t0 * P: t0 * P + full * P, :WEL]
                                .rearrange("(j p) w -> p j w", p=P),
                                in_=pkg2[:, :full, :WEL])
                        tail = rows - full * P
                        if tail:
                            nc.sync.dma_start(
                                out=tab2loc[t0 * P + full * P:
                                            t0 * P + full * P + tail, :WEL],
                                in_=pkg2[:tail, full, :WEL])
                    else:
                        nc.sync.dma_start(
                            out=out_ext[t0 * P: (t0 + gt) * P, :]
                            .rearrange("(j p) d -> p j d", p=P),
                            in_=o2g[:, :gt, :])

            for _rep in range(reps):
                phase_a1()
                build_er1()
                phase_b(0)
                nc.gpsimd.collective_compute(
                    "AllGather",
                    mybir.AluOpType.bypass,
                    replica_groups=[list(range(c.cores))],
                    ins=[tab2loc[:]],
                    outs=[tab2[:]],
                )
                phase_b(1)

    nc.compile()
    return nc


# ---------------------------------------------------------------- reference

def ref_np(inputs, cfg):
    c = cfg
    x = np.asarray(inputs["data"], np.float64)
    src = np.asarray(inputs["src"]).astype(np.int64)
    dst = np.asarray(inputs["dst"]).astype(np.int64)

    def layer(x, W, al, ar, b):
        N = x.shape[0]
        feat = (x @ np.asarray(W, np.float64)).reshape(N, c.H, c.HD)
        el = np.einsum("nhd,hd->nh", feat, np.asarray(al, np.float64))
        er = np.einsum("nhd,hd->nh", feat, np.asarray(ar, np.float64))
        e = el[src] + er[dst]
        e = np.where(e > 0, e, c.NEG * e)
        m = np.full((N, c.H), -np.inf)
        np.maximum.at(m, dst, e)
        a = np.exp(e - m[dst])
        den = np.zeros((N, c.H))
        np.add.at(den, dst, a)
        alpha = a / den[dst]
        msg = feat[src] * alpha[:, :, None]
        out = np.zeros((N, c.H, c.HD))
        np.add.at(out, dst, msg)
        out = out + np.asarray(b, np.float64).reshape(1, c.H, c.HD)
        return np.maximum(out, 0).reshape(N, c.D)

    h = layer(x, inputs["W1"], inputs["al1"], inputs["ar1"], inputs["b1"])
    h = layer(h, inputs["W2"], inputs["al2"], inputs["ar2"], inputs["b2"])
    return h


# ---------------------------------------------------------------- entry point

_BUILD_CACHE = {}


def kernel(**inputs) -> np.ndarray:
    """Full-input GAT kernel: shards internally across 8 NeuronCores."""
    from concourse.bass_utils import run_bass_kernel_spmd

    cfg = make_cfg(N=50000, E=800000, D=256, H=8, cores=8)
    in_maps, meta = prep_all(inputs, cfg)
    key = (meta.KA, meta.KB)
    if key not in _BUILD_CACHE:
        _BUILD_CACHE[key] = build_nc(cfg, meta.KA, meta.KB)
    nc = _BUILD_CACHE[key]
    res = run_bass_kernel_spmd(nc, in_maps, list(range(cfg.cores)))
    results = [{"out": res.results[ci]["out"]} for ci in range(cfg.cores)]
    out = finalize(results, cfg, meta)
    return np.ascontiguousarray(out.astype(np.float32))


# revision 6
# speedup vs baseline: 1.0110x; 1.0021x over previous
"""2-layer GAT (DGL GATConv style) on 8 trn2 NeuronCores via Bass/Tile. v2

Design:
  - Edges dst-sorted on host; cores own contiguous equal node ranges
    [c*NPC, (c+1)*NPC) and all edges whose dst falls in range.
  - Layer 1 phase A (replicated): packed table row (f32 container)
    [feat(D bf16) | el(H f32) | er(H f32) | pad] = x @ [W | Wal | War],
    written to tab1 in groups of 8 tiles (one slab load + one table write
    per group). Feature cols h-innermost (packed col j <-> head j%H).
  - er for own nodes lives in SBUF (er_tab[d, t, h], d = dst position in
    tile t). Layer 1 builds it with two small gathers (tabA/tabB halves,
    host-clipped indices) + mask merge; layer 2 captures it for free from
    phase-A PSUM (the layer-2 transform is sharded, tiles align).
  - Phase B (sharded): per dst-tile, gather src rows (768 B each) with
    dma_gather. S one-hots (slot->dst, for segment-sum matmuls) are
    generated in ONE 2x-rate DVE op per tile (layout [P, P, K] keeps every
    operand innermost-packed); M one-hots (dst->slot transposes, for the
    er broadcast) are host-built fp8 data streamed from DRAM (0/1 is exact
    in fp8; half the bytes of bf16). er per slot = K tiny M^T @ er_tab
    matmuls instead of a 256 B/edge gather. exp(leakyrelu(el+er)) ->
    weighted messages -> segment sum via S matmuls into PSUM [num | den];
    normalize, bias, relu.
  - Layer 2 phase A is fused into phase B of layer 1: each B0 tile's
    output hb is PE-transposed and immediately transformed (2 matmuls) to
    its packed table row; rows go to tab2loc [NPC, ROWF]. After B0 an
    AllGather concatenates the 8 slices into the full tab2 - no hT
    roundtrip and no replicated layer-2 transform.
"""

import math
import sys
from types import SimpleNamespace

import numpy as np

sys.path.insert(0, "/opt/trn_rl_repo")

from concourse import bacc, bass, mybir, tile  # noqa: E402

F32 = mybir.dt.float32
BF16 = mybir.dt.bfloat16
FP8 = mybir.dt.float8e4
I32 = mybir.dt.int32
I16 = mybir.dt.int16

P = 128


def make_cfg(N=50000, E=800000, D=256, H=8, cores=8, split=None):
    HD = D // H
    NPC = N // cores
    NT = math.ceil(NPC / P)          # dst tiles per core
    NTA = math.ceil(N / P)           # phase-A node tiles (layer 1, flat)
    ROWU = ((D + 4 * H + 127) // 128) * 128   # packed row, bf16 units (256B mult)
    if split is None:
        split = NPC * min(cores, 32767 // NPC)
        split = min(split, N)
    assert split % NPC == 0 and split <= 32767 + 1 and N - split <= 32767 + 1
    return SimpleNamespace(
        N=N, E=E, D=D, H=H, HD=HD, cores=cores, NPC=NPC, NT=NT, NTA=NTA,
        ROWU=ROWU, ROWF=ROWU // 2, SPLIT=split,
        G1=8, GB=7,
        NEG=0.2,
    )


# ---------------------------------------------------------------- host prep

def perm_h_inner(D, H):
    """perm[j] = original feature index stored at packed col j (h-innermost)."""
    HD = D // H
    j = np.arange(D)
    return (j % H) * HD + j // H


def attn_cols(W, a, H):
    D = W.shape[0]
    HD = W.shape[1] // H
    return np.stack(
        [W[:, h * HD:(h + 1) * HD] @ a[h] for h in range(H)], axis=1
    )


def wrap16(flat, reps=8):
    """[num] -> [16*reps, num//16] int16 wrapped layout (idx i at [i%16, i//16]),
    replicated across the 8 gpsimd cores."""
    num = len(flat)
    assert num % 16 == 0
    a = np.zeros((16, num // 16), dtype=np.int16)
    a[np.arange(num) % 16, np.arange(num) // 16] = flat
    return np.tile(a, (reps, 1))


def prep_edges(src, dst, cfg):
    c = cfg
    order = np.argsort(dst, kind="stable")
    src_s = src[order].astype(np.int64)
    dst_s = dst[order].astype(np.int64)

    core = dst_s // c.NPC
    loc = dst_s - core * c.NPC
    lt = loc // P
    pos = loc - lt * P
    islow = src_s < c.SPLIT

    # group = (core, tile, islow) ; edges of a group get consecutive slots
    g = (core * c.NT + lt) * 2 + (1 - islow)   # low first
    order2 = np.argsort(g, kind="stable")
    src_s, dst_s, core, lt, pos, islow, g = (
        a[order2] for a in (src_s, dst_s, core, lt, pos, islow, g))

    uniq, starts = np.unique(g, return_index=True)
    start_of = np.zeros(c.cores * c.NT * 2, dtype=np.int64)
    start_of[uniq] = starts
    rank = np.arange(len(dst_s)) - start_of[g]

    counts = np.zeros(c.cores * c.NT * 2, dtype=np.int64)
    np.add.at(counts, g, 1)
    KA = int(math.ceil(counts[0::2].max() / P))
    KB = int(math.ceil(counts[1::2].max() / P)) if counts[1::2].max() > 0 else 0
    K = KA + KB

    # slot within tile: low edges fill chunks [0,KA), high fill [KA,K)
    chunk = rank // P + np.where(islow, 0, KA)
    part = rank % P

    srcA = np.zeros((c.cores, c.NT * KA * P), dtype=np.int64)
    srcB = np.zeros((c.cores, c.NT * KB * P), dtype=np.int64) if KB else None
    dstpos = np.full((c.cores, P, c.NT * K), 255.0, dtype=np.float32)
    dstposT = np.full((c.cores, c.NT * K * P), 255, dtype=np.int16)

    iA = lt * (KA * P) + (chunk * P + part)            # valid where islow
    iB = lt * (KB * P) + ((chunk - KA) * P + part) if KB else None
    low = islow
    srcA[core[low], iA[low]] = src_s[low]
    if KB:
        hi = ~islow
        srcB[core[hi], iB[hi]] = src_s[hi] - c.SPLIT
    dstpos[core, part, lt * K + chunk] = pos
    dstposT[core, (lt * K + chunk) * P + part] = pos

    # host-built transposed one-hots M[d, (t, ck, p)] = (dstposT == d).
    # fp8e4m3 is exact for 0/1 and halves the stream bytes.
    import ml_dtypes
    Mhot = (dstposT[:, None, :] == np.arange(P, dtype=np.int16)[None, :, None]
            ).astype(ml_dtypes.float8_e4m3fn)

    srcA16 = np.stack([wrap16(srcA[ci]) for ci in range(c.cores)])
    srcB16 = (np.stack([wrap16(srcB[ci]) for ci in range(c.cores)])
              if KB else np.zeros((c.cores, P, 0), np.int16))
    return srcA16, srcB16, dstpos, Mhot, KA, KB


def prep_all(inputs, cfg):
    c = cfg
    perm = perm_h_inner(c.D, c.H)
    x = np.asarray(inputs["data"], np.float32)
    src = np.asarray(inputs["src"]).astype(np.int64)
    dst = np.asarray(inputs["dst"]).astype(np.int64)

    def rhs_for(W, al, ar, permute_rows):
        W = np.asarray(W, np.float64)
        Wal = attn_cols(W, np.asarray(al, np.float64), c.H)
        War = attn_cols(W, np.asarray(ar, np.float64), c.H)
        Wp = W[:, perm]
        if permute_rows:
            Wp, Wal, War = Wp[perm], Wal[perm], War[perm]
        return to_bf16(np.concatenate([Wp, Wal, War], axis=1))

    rhs1 = rhs_for(inputs["W1"], inputs["al1"], inputs["ar1"], False)
    rhs2 = rhs_for(inputs["W2"], inputs["al2"], inputs["ar2"], True)
    b1 = np.asarray(inputs["b1"], np.float32)[perm].reshape(1, c.D)
    b2 = np.asarray(inputs["b2"], np.float32)[perm].reshape(1, c.D)
    xT = to_bf16(x.T.copy())

    srcA16, srcB16, dstpos, Mhot, KA, KB = prep_edges(src, dst, c)

    # er build indices (layer 1): row i (i in [0, NT*128)) <- own node NPC*ci+i
    erbA, erbB, masks = [], [], []
    for ci in range(c.cores):
        base = ci * c.NPC
        rows = np.arange(c.NT * P)
        ra = np.where(base < c.SPLIT, base + rows, 0)
        ra = np.clip(ra, 0, c.SPLIT - 1)
        rb = np.where(base >= c.SPLIT, base - c.SPLIT + rows, 0)
        rb = np.clip(rb, 0, max(c.N - c.SPLIT - 1, 0))
        erbA.append(wrap16(ra))
        erbB.append(wrap16(rb))
        mA = 1.0 if base < c.SPLIT else 0.0
        m = np.zeros((P, 2), np.float32)
        m[:, 0] = mA
        m[:, 1] = 1.0 - mA
        masks.append(m)

    meta = SimpleNamespace(perm=perm, KA=KA, KB=KB, K=KA + KB)
    in_maps = []
    for ci in range(c.cores):
        in_maps.append({
            "xT": xT, "rhs1": rhs1, "rhs2": rhs2, "b1row": b1, "b2row": b2,
            "srcA16": srcA16[ci], "srcB16": srcB16[ci],
            "dstpos": to_bf16(dstpos[ci]),
            "Mhot": Mhot[ci],
            "erbA16": erbA[ci], "erbB16": erbB[ci],
            "maskAB": masks[ci],
        })
    return in_maps, meta


def to_bf16(a):
    import ml_dtypes
    return np.asarray(a).astype(ml_dtypes.bfloat16)


def finalize(results, cfg, meta):
    c = cfg
    parts = [results[ci]["out"][: c.NPC] for ci in range(c.cores)]
    out_p = np.concatenate(parts, axis=0)
    out = np.empty_like(out_p)
    out[:, meta.perm] = out_p
    return out


# ---------------------------------------------------------------- kernel

def build_nc(cfg, KA, KB, debug=False, reps=1):
    c = cfg
    K = KA + KB
    ERR = c.NT * P                   # er rows per core (padded)
    NPAD = c.NTA * P                 # padded table rows (layer 1)
    DH = c.D + c.H                   # seg-matmul rhs cols (msg + den)
    D2H = c.D + 2 * c.H              # phase-A rhs cols
    WC = c.D // 2 + 2 * c.H          # written f32 cols of a packed row

    nc = bacc.Bacc("TRN2", target_bir_lowering=False, debug=debug,
                   num_devices=c.cores, num_swdge_queues=2)

    xT = nc.declare_dram_parameter("xT", [c.D, c.N], BF16, isOutput=False)
    rhs1 = nc.declare_dram_parameter("rhs1", [c.D, D2H], BF16, isOutput=False)
    rhs2 = nc.declare_dram_parameter("rhs2", [c.D, D2H], BF16, isOutput=False)
    b1row = nc.declare_dram_parameter("b1row", [1, c.D], F32, isOutput=False)
    b2row = nc.declare_dram_parameter("b2row", [1, c.D], F32, isOutput=False)
    srcA16 = nc.declare_dram_parameter("srcA16", [P, c.NT * KA * 8], I16, isOutput=False)
    srcB16 = nc.declare_dram_parameter("srcB16", [P, c.NT * KB * 8], I16, isOutput=False)
    dstpos = nc.declare_dram_parameter("dstpos", [P, c.NT * K], BF16, isOutput=False)
    Mhot = nc.declare_dram_parameter("Mhot", [P, c.NT * K * P], FP8, isOutput=False)
    erbA16 = nc.declare_dram_parameter("erbA16", [P, ERR // 16], I16, isOutput=False)
    erbB16 = nc.declare_dram_parameter("erbB16", [P, ERR // 16], I16, isOutput=False)
    maskAB = nc.declare_dram_parameter("maskAB", [P, 2], F32, isOutput=False)
    out_ext = nc.declare_dram_parameter("out", [c.NT * P, c.D], F32, isOutput=True)

    tab1 = nc.dram_tensor("tab1", [NPAD, c.ROWF], F32)
    tab2loc = nc.dram_tensor("tab2loc", [c.NPC, c.ROWF], F32)
    tab2 = nc.dram_tensor("tab2", [c.N, c.ROWF], F32,
                          addr_space="Shared" if c.cores > 4 else "Local")

    with tile.TileContext(nc) as tc:
        with (
            tc.tile_pool(name="const", bufs=1) as constp,
            tc.tile_pool(name="slab", bufs=3) as slabp,
            tc.tile_pool(name="pkg", bufs=2) as pkgp,
            tc.tile_pool(name="eb", bufs=1) as ebp,
            tc.tile_pool(name="ertab", bufs=1) as ertabp,
            tc.tile_pool(name="gath", bufs=4) as gathp,
            tc.tile_pool(name="onehot", bufs=3) as onehotp,
            tc.tile_pool(name="sel", bufs=3) as selp,
            tc.tile_pool(name="rhsm", bufs=2) as rhsmp,
            tc.tile_pool(name="small", bufs=3) as smallp,
            tc.tile_pool(name="lt2", bufs=3) as lt2p,
            tc.tile_pool(name="outp", bufs=2) as outp,
            tc.tile_pool(name="psA", bufs=4, space="PSUM") as psA,
            tc.tile_pool(name="psB", bufs=2, space="PSUM") as psB,
            tc.tile_pool(name="psE", bufs=1, space="PSUM") as psE,
            tc.tile_pool(name="psT", bufs=1, space="PSUM") as psT,
        ):
            # ---------------- constants
            iota = constp.tile([P, P], BF16, tag="iota")
            nc.gpsimd.iota(iota[:], [[1, P]], channel_multiplier=0,
                           allow_small_or_imprecise_dtypes=True)
            iota_kp = constp.tile([P, P, K], BF16, tag="iota_kp")
            nc.gpsimd.iota(iota_kp[:], [[1, P], [0, K]], channel_multiplier=0,
                           allow_small_or_imprecise_dtypes=True)
            from concourse.masks import make_identity
            ident = constp.tile([P, P], BF16, tag="ident")
            make_identity(nc, ident[:])

            def load_const(name, param, shape, dt):
                t = constp.tile(shape, dt, tag=name, name=name)
                nc.sync.dma_start(out=t[:], in_=param[:, :])
                return t

            srcA_sb = load_const("srcA_sb", srcA16, [P, c.NT * KA * 8], I16)
            srcB_sb = load_const("srcB_sb", srcB16, [P, c.NT * KB * 8], I16)
            dstpos_sb = load_const("dstpos_sb", dstpos, [P, c.NT * K], BF16)
            erbA_sb = load_const("erbA_sb", erbA16, [P, ERR // 16], I16)
            erbB_sb = load_const("erbB_sb", erbB16, [P, ERR // 16], I16)
            mask_sb = load_const("mask_sb", maskAB, [P, 2], F32)

            rhsW = [[constp.tile([P, D2H], BF16,
                                 tag=f"rhsW{l}_{kb}", name=f"rhsW{l}_{kb}")
                     for kb in range(2)] for l in range(2)]
            for l, rt in enumerate([rhs1, rhs2]):
                for kb in range(2):
                    nc.sync.dma_start(out=rhsW[l][kb][:],
                                      in_=rt[kb * P: (kb + 1) * P, :])
            b_bc = [constp.tile([P, c.D], F32, tag=f"bbc{l}", name=f"bbc{l}")
                    for l in range(2)]
            for l, bt in enumerate([b1row, b2row]):
                nc.sync.dma_start(out=b_bc[l][:],
                                  in_=bt[0:1, :].to_broadcast([P, c.D]))

            # Pre-touch consts on compute engines so first uses don't carry
            # extra sync waits on deep pipelines.
            warm = constp.tile([P, 4], F32, tag="warm")
            warmb = warm[:].bitcast(BF16)
            nc.vector.tensor_copy(out=warm[:, 0:1], in_=dstpos_sb[:, 0:1])
            nc.vector.tensor_copy(out=warmb[:, 2:3], in_=iota[:, 0:1])
            nc.vector.tensor_copy(out=warm[:, 1:2], in_=mask_sb[:, 0:1])

            er_tab = [ertabp.tile([P, c.NT, c.H], BF16, tag=f"ertab{l}",
                                  name=f"ertab{l}")
                      for l in range(2)]

            # ---------------- phase A layer 1 (replicated, batched groups)
            def phase_a1():
                ngrp = math.ceil(c.NTA / c.G1)
                for grp in range(ngrp):
                    t0 = grp * c.G1
                    gt = min(c.G1, c.NTA - t0)
                    cols = min(gt * P, c.N - t0 * P)
                    slab = slabp.tile([P, 2, c.G1 * P], BF16, tag="slab")
                    nc.scalar.dma_start(
                        out=slab[:, :, :cols],
                        in_=xT[:, t0 * P: t0 * P + cols]
                        .rearrange("(kb p) n -> p kb n", p=P))
                    pkg = pkgp.tile([P, c.G1, c.ROWF], F32, tag="pkg")
                    pkgb = pkg[:].bitcast(BF16)
                    for j in range(gt):
                        m = min(P, c.N - (t0 + j) * P)
                        ps = psA.tile([P, D2H], F32, tag="psA")
                        for kb in range(2):
                            nc.tensor.matmul(
                                out=ps[:m, :],
                                lhsT=slab[:, kb, j * P: j * P + m],
                                rhs=rhsW[0][kb][:],
                                start=(kb == 0), stop=(kb == 1))
                        if j % 2 == 0:
                            nc.scalar.copy(out=pkgb[:m, j, : c.D],
                                           in_=ps[:m, : c.D])
                            nc.vector.tensor_copy(
                                out=pkg[:m, j, c.D // 2: WC],
                                in_=ps[:m, c.D: D2H])
                        else:
                            nc.vector.tensor_copy(out=pkgb[:m, j, : c.D],
                                                  in_=ps[:m, : c.D])
                            nc.scalar.copy(
                                out=pkg[:m, j, c.D // 2: WC],
                                in_=ps[:m, c.D: D2H])
                    nc.sync.dma_start(
                        out=tab1[t0 * P: (t0 + gt) * P, :WC]
                        .rearrange("(j p) w -> p j w", p=P),
                        in_=pkg[:, :gt, :WC])

            # ---------------- er_tab build for layer 1 (2 gathers + merge)
            def build_er1():
                start = c.ROWF - 64
                eroff = (c.D // 2 + c.H) - start
                half_nt = (c.NT + 1) // 2
                for piece in range(2):
                    tlo = piece * half_nt
                    tn = min(half_nt, c.NT - tlo)
                    ebs = []
                    for half, (r0, r1, idx_sb) in enumerate([
                            (0, c.SPLIT, erbA_sb),
                            (c.SPLIT, NPAD, erbB_sb)]):
                        eb = ebp.tile([P, half_nt, 64], F32,
                                      tag=f"eb{half}", name=f"eb{half}")
                        nc.gpsimd.dma_gather(
                            out_ap=eb[:, :tn, :],
                            in_ap=tab1[r0:r1, start: start + 64],
                            idxs_ap=idx_sb[:, tlo * 8: (tlo + tn) * 8],
                            num_idxs=tn * P, num_idxs_reg=tn * P,
                            elem_size=64, elem_step=c.ROWF,
                            single_packet=False, queue_num=half)
                        ebs.append(eb)
                    tmp = smallp.tile([P, half_nt, c.H], F32, tag="ermerge")
                    nc.vector.tensor_scalar(
                        out=tmp[:, :tn], in0=ebs[0][:, :tn, eroff:eroff + c.H],
                        scalar1=mask_sb[:, 0:1], scalar2=None,
                        op0=mybir.AluOpType.mult)
                    nc.vector.scalar_tensor_tensor(
                        out=er_tab[0][:, tlo: tlo + tn],
                        in0=ebs[1][:, :tn, eroff:eroff + c.H],
                        scalar=mask_sb[:, 1:2], in1=tmp[:, :tn],
                        op0=mybir.AluOpType.mult, op1=mybir.AluOpType.add)

            # ---------------- phase B (layer = 0 or 1)
            def phase_b(layer):
                tabA0 = tab1 if layer == 0 else tab2
                ngrp = math.ceil(c.NT / c.GB)
                for grp in range(ngrp):
                    t0 = grp * c.GB
                    gt = min(c.GB, c.NT - t0)
                    if layer == 0:
                        pkg2 = pkgp.tile([P, c.GB, c.ROWF], F32, tag="pkg2")
                        pkg2b = pkg2[:].bitcast(BF16)
                    else:
                        o2g = outp.tile([P, c.GB, c.D], F32, tag="o2g")
                    for j in range(gt):
                        t = t0 + j
                        g = gathp.tile([P, K, c.ROWF], F32, tag="gath")
                        nc.gpsimd.dma_gather(
                            out_ap=g[:, 0:KA, :], in_ap=tabA0[0: c.SPLIT, :],
                            idxs_ap=srcA_sb[:, t * KA * 8:(t + 1) * KA * 8],
                            num_idxs=KA * P, num_idxs_reg=KA * P,
                            elem_size=c.ROWF, single_packet=KA * P <= 1024)
                        nc.gpsimd.dma_gather(
                            out_ap=g[:, KA:K, :],
                            in_ap=tabA0[c.SPLIT: c.N, :],
                            idxs_ap=srcB_sb[:, t * KB * 8:(t + 1) * KB * 8],
                            num_idxs=KB * P, num_idxs_reg=KB * P,
                            elem_size=c.ROWF, single_packet=KB * P <= 1024,
                            queue_num=1)
                        gb = g[:].bitcast(BF16)

                        # transposed one-hots for this tile (host data)
                        M_all = onehotp.tile([P, K, P], FP8, tag="M")
                        nc.scalar.dma_start(
                            out=M_all[:],
                            in_=Mhot[:, t * K * P: (t + 1) * K * P]
                            .rearrange("p (k q) -> p k q", k=K))

                        # er per slot: K tiny matmuls M_all[:, ck, :]^T @ er_tab
                        erp = psE.tile([P, K * c.H], F32, tag="psE")
                        for ck in range(K):
                            nc.tensor.matmul(
                                out=erp[:, ck * c.H:(ck + 1) * c.H],
                                lhsT=M_all[:, ck, :],
                                rhs=er_tab[layer][:, t, :],
                                start=True, stop=True)

                        # e = leakyrelu(el + er); exp
                        ea = smallp.tile([P, K, c.H], F32, tag="eadd")
                        nc.vector.tensor_tensor(
                            out=ea[:], in0=g[:, :, c.D // 2: c.D // 2 + c.H],
                            in1=erp[:].rearrange("p (k h) -> p k h", h=c.H),
                            op=mybir.AluOpType.add)
                        lr = smallp.tile([P, K, c.H], F32, tag="lrout")
                        nc.vector.scalar_tensor_tensor(
                            out=lr[:], in0=ea[:], scalar=c.NEG, in1=ea[:],
                            op0=mybir.AluOpType.mult, op1=mybir.AluOpType.max)
                        rm = rhsmp.tile([P, K, DH], BF16, tag="rhsm")
                        nc.scalar.activation(
                            out=rm[:, :, c.D: DH], in_=lr[:],
                            func=mybir.ActivationFunctionType.Exp)
                        expb = rm[:, :, c.D: DH].unsqueeze(2).to_broadcast(
                            [P, K, c.HD, c.H])
                        feat4 = gb[:, :, : c.D].rearrange(
                            "p k (hd h) -> p k hd h", h=c.H)
                        out4 = rm[:, :, : c.D].rearrange(
                            "p k (hd h) -> p k hd h", h=c.H)
                        nc.vector.tensor_tensor(out=out4, in0=feat4, in1=expb,
                                                op=mybir.AluOpType.mult)

                        # segment sum via one-hot matmuls; all K S-chunks
                        # generated in one 2x-eligible DVE op (layout [P,P,K])
                        S_allT = selp.tile([P, P, K], BF16, tag="S_allT")
                        nc.vector.tensor_tensor(
                            out=S_allT[:],
                            in0=iota_kp[:],
                            in1=dstpos_sb[:, t * K:(t + 1) * K]
                            .unsqueeze(1).to_broadcast([P, P, K]),
                            op=mybir.AluOpType.is_equal)
                        ps = psB.tile([P, DH], F32, tag="psB")
                        for ck in range(K):
                            nc.tensor.matmul(out=ps[:], lhsT=S_allT[:, :, ck],
                                             rhs=rm[:, ck, :],
                                             start=(ck == 0), stop=(ck == K - 1))
                        den = smallp.tile([P, c.H], F32, tag="den")
                        nc.vector.tensor_scalar_max(den[:], ps[:, c.D: DH], 1e-30)
                        rcp = smallp.tile([P, c.H], F32, tag="rcp")
                        nc.vector.reciprocal(rcp[:], den[:])
                        o1 = outp.tile([P, c.D], F32, tag="o1")
                        rcpb = rcp[:].unsqueeze(1).to_broadcast([P, c.HD, c.H])
                        ps4 = ps[:, : c.D].rearrange("p (hd h) -> p hd h", h=c.H)
                        o14 = o1[:].rearrange("p (hd h) -> p hd h", h=c.H)
                        nc.vector.tensor_tensor(out=o14, in0=ps4, in1=rcpb,
                                                op=mybir.AluOpType.mult)
                        nc.vector.tensor_tensor(out=o1[:], in0=o1[:],
                                                in1=b_bc[layer][:],
                                                op=mybir.AluOpType.add)
                        if layer == 0:
                            # h for this tile -> layer-2 phase A (sharded)
                            hb = outp.tile([P, c.D], BF16, tag="hb")
                            nc.vector.tensor_scalar_max(hb[:], o1[:], 0.0)
                            ps2 = psA.tile([P, D2H], F32, tag="psA")
                            for kb in range(2):
                                pst = psT.tile([P, P], BF16, tag="psT")
                                nc.tensor.transpose(
                                    out=pst[:],
                                    in_=hb[:, kb * P: (kb + 1) * P],
                                    identity=ident[:])
                                lt2 = lt2p.tile([P, P], BF16, tag="lt2")
                                nc.scalar.copy(out=lt2[:], in_=pst[:])
                                nc.tensor.matmul(
                                    out=ps2[:], lhsT=lt2[:],
                                    rhs=rhsW[1][kb][:],
                                    start=(kb == 0), stop=(kb == 1))
                            nc.scalar.copy(out=pkg2b[:, j, : c.D],
                                           in_=ps2[:, : c.D])
                            nc.scalar.copy(
                                out=pkg2[:, j, c.D // 2: c.D // 2 + c.H],
                                in_=ps2[:, c.D: c.D + c.H])
                            nc.scalar.copy(
                                out=er_tab[1][:, t, :],
                                in_=ps2[:, c.D + c.H: D2H])
                        else:
                            nc.vector.tensor_scalar_max(o2g[:, j, :], o1[:], 0.0)
                    # group epilogue
                    if layer == 0:
                        rows = min(gt * P, c.NPC - t0 * P)
                        full = rows // P
                        WEL = c.D // 2 + c.H
                        if full:
                            nc.sync.dma_start(
                                out=tab2loc[t0 * P: t0 * P + full * P, :WEL]
                                .rearrange("(j p) w -> p j w", p=P),
                                in_=pkg2[:, :full, :WEL])
                        tail = rows - full * P
                        if tail:
                            nc.sync.dma_start(
                                out=tab2loc[t0 * P + full * P:
                                            t0 * P + full * P + tail, :WEL],
                                in_=pkg2[:tail, full, :WEL])
                    else:
                        nc.sync.dma_start(
                            out=out_ext[t0 * P: (t0 + gt) * P, :]
                            .rearrange("(j p) d -> p j d", p=P),
                            in_=o2g[:, :gt, :])

            for _rep in range(reps):
                phase_a1()
                build_er1()
                phase_b(0)
                nc.gpsimd.collective_compute(
                    "AllGather",
                    mybir.AluOpType.bypass,
                    replica_groups=[list(range(c.cores))],
                    ins=[tab2loc[:]],
                    outs=[tab2[:]],
                )
                phase_b(1)

    nc.compile()
    return nc


# ---------------------------------------------------------------- reference

def ref_np(inputs, cfg):
    c = cfg
    x = np.asarray(inputs["data"], np.float64)
    src = np.asarray(inputs["src"]).astype(np.int64)
    dst = np.asarray(inputs["dst"]).astype(np.int64)

    def layer(x, W, al, ar, b):
        N = x.shape[0]
        feat = (x @ np.asarray(W, np.float64)).reshape(N, c.H, c.HD)
        el = np.einsum("nhd,hd->nh", feat, np.asarray(al, np.float64))
        er = np.einsum("nhd,hd->nh", feat, np.asarray(ar, np.float64))
        e = el[src] + er[dst]
        e = np.where(e > 0, e, c.NEG * e)
        m = np.full((N, c.H), -np.inf)
        np.maximum.at(m, dst, e)
        a = np.exp(e - m[dst])
        den = np.zeros((N, c.H))
        np.add.at(den, dst, a)
        alpha = a / den[dst]
        msg = feat[src] * alpha[:, :, None]
        out = np.zeros((N, c.H, c.HD))
        np.add.at(out, dst, msg)
        out = out + np.asarray(b, np.float64).reshape(1, c.H, c.HD)
        return np.maximum(out, 0).reshape(N, c.D)

    h = layer(x, inputs["W1"], inputs["al1"], inputs["ar1"], inputs["b1"])
    h = layer(h, inputs["W2"], inputs["al2"], inputs["ar2"], inputs["b2"])
    return h


# ---------------------------------------------------------------- entry point

_BUILD_CACHE = {}


def kernel(**inputs) -> np.ndarray:
    """Full-input GAT kernel: shards internally across 8 NeuronCores."""
    from concourse.bass_utils import run_bass_kernel_spmd

    cfg = make_cfg(N=50000, E=800000, D=256, H=8, cores=8)
    in_maps, meta = prep_all(inputs, cfg)
    key = (meta.KA, meta.KB)
    if key not in _BUILD_CACHE:
        _BUILD_CACHE[key] = build_nc(cfg, meta.KA, meta.KB)
    nc = _BUILD_CACHE[key]
    res = run_bass_kernel_spmd(nc, in_maps, list(range(cfg.cores)))
    results = [{"out": res.results[ci]["out"]} for ci in range(cfg.cores)]
    out = finalize(results, cfg, meta)
    return np.ascontiguousarray(out.astype(np.float32))


# revision 7
# speedup vs baseline: 1.0116x; 1.0005x over previous
"""2-layer GAT (DGL GATConv style) on 8 trn2 NeuronCores via Bass/Tile. v2

Design:
  - Edges dst-sorted on host; cores own contiguous equal node ranges
    [c*NPC, (c+1)*NPC) and all edges whose dst falls in range.
  - Layer 1 phase A (replicated): packed table row (f32 container)
    [feat(D bf16) | el(H f32) | er(H f32) | pad] = x @ [W | Wal | War],
    written to tab1 in groups of 8 tiles (one slab load + one table write
    per group). Feature cols h-innermost (packed col j <-> head j%H).
  - er for own nodes lives in SBUF (er_tab[d, t, h], d = dst position in
    tile t). Layer 1 builds it with two small gathers (tabA/tabB halves,
    host-clipped indices) + mask merge; layer 2 captures it for free from
    phase-A PSUM (the layer-2 transform is sharded, tiles align).
  - Phase B (sharded): per dst-tile, gather src rows (768 B each) with
    dma_gather. S one-hots (slot->dst, for segment-sum matmuls) are
    generated in ONE 2x-rate DVE op per tile (layout [P, P, K] keeps every
    operand innermost-packed); M one-hots (dst->slot transposes, for the
    er broadcast) are host-built fp8 data streamed from DRAM (0/1 is exact
    in fp8; half the bytes of bf16). er per slot = K tiny M^T @ er_tab
    matmuls instead of a 256 B/edge gather. exp(leakyrelu(el+er)) ->
    weighted messages -> segment sum via S matmuls into PSUM [num | den];
    normalize, bias, relu.
  - Layer 2 phase A is fused into phase B of layer 1: each B0 tile's
    output hb is PE-transposed and immediately transformed (2 matmuls) to
    its packed table row; rows go to tab2loc [NPC, ROWF]. After B0 an
    AllGather concatenates the 8 slices into the full tab2 - no hT
    roundtrip and no replicated layer-2 transform.
"""

import math
import sys
from types import SimpleNamespace

import numpy as np

sys.path.insert(0, "/opt/trn_rl_repo")

from concourse import bacc, bass, mybir, tile  # noqa: E402

F32 = mybir.dt.float32
BF16 = mybir.dt.bfloat16
FP8 = mybir.dt.float8e4
I32 = mybir.dt.int32
I16 = mybir.dt.int16

P = 128


def make_cfg(N=50000, E=800000, D=256, H=8, cores=8, split=None):
    HD = D // H
    NPC = N // cores
    NT = math.ceil(NPC / P)          # dst tiles per core
    NTA = math.ceil(N / P)           # phase-A node tiles (layer 1, flat)
    ROWU = ((D + 4 * H + 127) // 128) * 128   # packed row, bf16 units (256B mult)
    if split is None:
        split = NPC * min(cores, 32767 // NPC)
        split = min(split, N)
    assert split % NPC == 0 and split <= 32767 + 1 and N - split <= 32767 + 1
    return SimpleNamespace(
        N=N, E=E, D=D, H=H, HD=HD, cores=cores, NPC=NPC, NT=NT, NTA=NTA,
        ROWU=ROWU, ROWF=ROWU // 2, SPLIT=split,
        G1=8, GB=7,
        NEG=0.2,
    )


# ---------------------------------------------------------------- host prep

def perm_h_inner(D, H):
    """perm[j] = original feature index stored at packed col j (h-innermost)."""
    HD = D // H
    j = np.arange(D)
    return (j % H) * HD + j // H


def attn_cols(W, a, H):
    D = W.shape[0]
    HD = W.shape[1] // H
    return np.stack(
        [W[:, h * HD:(h + 1) * HD] @ a[h] for h in range(H)], axis=1
    )


def wrap16(flat, reps=8):
    """[num] -> [16*reps, num//16] int16 wrapped layout (idx i at [i%16, i//16]),
    replicated across the 8 gpsimd cores."""
    num = len(flat)
    assert num % 16 == 0
    a = np.zeros((16, num // 16), dtype=np.int16)
    a[np.arange(num) % 16, np.arange(num) // 16] = flat
    return np.tile(a, (reps, 1))


def prep_edges(src, dst, cfg):
    c = cfg
    order = np.argsort(dst, kind="stable")
    src_s = src[order].astype(np.int64)
    dst_s = dst[order].astype(np.int64)

    core = dst_s // c.NPC
    loc = dst_s - core * c.NPC
    lt = loc // P
    pos = loc - lt * P
    islow = src_s < c.SPLIT

    # group = (core, tile, islow) ; edges of a group get consecutive slots
    g = (core * c.NT + lt) * 2 + (1 - islow)   # low first
    order2 = np.argsort(g, kind="stable")
    src_s, dst_s, core, lt, pos, islow, g = (
        a[order2] for a in (src_s, dst_s, core, lt, pos, islow, g))

    uniq, starts = np.unique(g, return_index=True)
    start_of = np.zeros(c.cores * c.NT * 2, dtype=np.int64)
    start_of[uniq] = starts
    rank = np.arange(len(dst_s)) - start_of[g]

    counts = np.zeros(c.cores * c.NT * 2, dtype=np.int64)
    np.add.at(counts, g, 1)
    KA = int(math.ceil(counts[0::2].max() / P))
    KB = int(math.ceil(counts[1::2].max() / P)) if counts[1::2].max() > 0 else 0
    K = KA + KB

    # slot within tile: low edges fill chunks [0,KA), high fill [KA,K)
    chunk = rank // P + np.where(islow, 0, KA)
    part = rank % P

    srcA = np.zeros((c.cores, c.NT * KA * P), dtype=np.int64)
    srcB = np.zeros((c.cores, c.NT * KB * P), dtype=np.int64) if KB else None
    dstpos = np.full((c.cores, P, c.NT * K), 255.0, dtype=np.float32)
    dstposT = np.full((c.cores, c.NT * K * P), 255, dtype=np.int16)

    iA = lt * (KA * P) + (chunk * P + part)            # valid where islow
    iB = lt * (KB * P) + ((chunk - KA) * P + part) if KB else None
    low = islow
    srcA[core[low], iA[low]] = src_s[low]
    if KB:
        hi = ~islow
        srcB[core[hi], iB[hi]] = src_s[hi] - c.SPLIT
    dstpos[core, part, lt * K + chunk] = pos
    dstposT[core, (lt * K + chunk) * P + part] = pos

    # host-built transposed one-hots M[d, (t, ck, p)] = (dstposT == d).
    # fp8e4m3 is exact for 0/1 and halves the stream bytes.
    import ml_dtypes
    Mhot = (dstposT[:, None, :] == np.arange(P, dtype=np.int16)[None, :, None]
            ).astype(ml_dtypes.float8_e4m3fn)

    srcA16 = np.stack([wrap16(srcA[ci]) for ci in range(c.cores)])
    srcB16 = (np.stack([wrap16(srcB[ci]) for ci in range(c.cores)])
              if KB else np.zeros((c.cores, P, 0), np.int16))
    return srcA16, srcB16, dstpos, Mhot, KA, KB


def prep_all(inputs, cfg):
    c = cfg
    perm = perm_h_inner(c.D, c.H)
    x = np.asarray(inputs["data"], np.float32)
    src = np.asarray(inputs["src"]).astype(np.int64)
    dst = np.asarray(inputs["dst"]).astype(np.int64)

    def rhs_for(W, al, ar, permute_rows):
        W = np.asarray(W, np.float64)
        Wal = attn_cols(W, np.asarray(al, np.float64), c.H)
        War = attn_cols(W, np.asarray(ar, np.float64), c.H)
        Wp = W[:, perm]
        if permute_rows:
            Wp, Wal, War = Wp[perm], Wal[perm], War[perm]
        return to_bf16(np.concatenate([Wp, Wal, War], axis=1))

    rhs1 = rhs_for(inputs["W1"], inputs["al1"], inputs["ar1"], False)
    rhs2 = rhs_for(inputs["W2"], inputs["al2"], inputs["ar2"], True)
    b1 = np.asarray(inputs["b1"], np.float32)[perm].reshape(1, c.D)
    b2 = np.asarray(inputs["b2"], np.float32)[perm].reshape(1, c.D)
    xT = to_bf16(x.T.copy())

    srcA16, srcB16, dstpos, Mhot, KA, KB = prep_edges(src, dst, c)

    # er build indices (layer 1): row i (i in [0, NT*128)) <- own node NPC*ci+i
    erbA, erbB, masks = [], [], []
    for ci in range(c.cores):
        base = ci * c.NPC
        rows = np.arange(c.NT * P)
        ra = np.where(base < c.SPLIT, base + rows, 0)
        ra = np.clip(ra, 0, c.SPLIT - 1)
        rb = np.where(base >= c.SPLIT, base - c.SPLIT + rows, 0)
        rb = np.clip(rb, 0, max(c.N - c.SPLIT - 1, 0))
        erbA.append(wrap16(ra))
        erbB.append(wrap16(rb))
        mA = 1.0 if base < c.SPLIT else 0.0
        m = np.zeros((P, 2), np.float32)
        m[:, 0] = mA
        m[:, 1] = 1.0 - mA
        masks.append(m)

    meta = SimpleNamespace(perm=perm, KA=KA, KB=KB, K=KA + KB)
    in_maps = []
    for ci in range(c.cores):
        in_maps.append({
            "xT": xT, "rhs1": rhs1, "rhs2": rhs2, "b1row": b1, "b2row": b2,
            "srcA16": srcA16[ci], "srcB16": srcB16[ci],
            "dstpos": to_bf16(dstpos[ci]),
            "Mhot": Mhot[ci],
            "erbA16": erbA[ci], "erbB16": erbB[ci],
            "maskAB": masks[ci],
        })
    return in_maps, meta


def to_bf16(a):
    import ml_dtypes
    return np.asarray(a).astype(ml_dtypes.bfloat16)


def finalize(results, cfg, meta):
    c = cfg
    parts = [results[ci]["out"][: c.NPC] for ci in range(c.cores)]
    out_p = np.concatenate(parts, axis=0)
    out = np.empty_like(out_p)
    out[:, meta.perm] = out_p
    return out


# ---------------------------------------------------------------- kernel

def build_nc(cfg, KA, KB, debug=False, reps=1):
    c = cfg
    K = KA + KB
    ERR = c.NT * P                   # er rows per core (padded)
    NPAD = c.NTA * P                 # padded table rows (layer 1)
    DH = c.D + c.H                   # seg-matmul rhs cols (msg + den)
    D2H = c.D + 2 * c.H              # phase-A rhs cols
    WC = c.D // 2 + 2 * c.H          # written f32 cols of a packed row

    nc = bacc.Bacc("TRN2", target_bir_lowering=False, debug=debug,
                   num_devices=c.cores, num_swdge_queues=2)

    xT = nc.declare_dram_parameter("xT", [c.D, c.N], BF16, isOutput=False)
    rhs1 = nc.declare_dram_parameter("rhs1", [c.D, D2H], BF16, isOutput=False)
    rhs2 = nc.declare_dram_parameter("rhs2", [c.D, D2H], BF16, isOutput=False)
    b1row = nc.declare_dram_parameter("b1row", [1, c.D], F32, isOutput=False)
    b2row = nc.declare_dram_parameter("b2row", [1, c.D], F32, isOutput=False)
    srcA16 = nc.declare_dram_parameter("srcA16", [P, c.NT * KA * 8], I16, isOutput=False)
    srcB16 = nc.declare_dram_parameter("srcB16", [P, c.NT * KB * 8], I16, isOutput=False)
    dstpos = nc.declare_dram_parameter("dstpos", [P, c.NT * K], BF16, isOutput=False)
    Mhot = nc.declare_dram_parameter("Mhot", [P, c.NT * K * P], FP8, isOutput=False)
    erbA16 = nc.declare_dram_parameter("erbA16", [P, ERR // 16], I16, isOutput=False)
    erbB16 = nc.declare_dram_parameter("erbB16", [P, ERR // 16], I16, isOutput=False)
    maskAB = nc.declare_dram_parameter("maskAB", [P, 2], F32, isOutput=False)
    out_ext = nc.declare_dram_parameter("out", [c.NT * P, c.D], F32, isOutput=True)

    tab1A = nc.dram_tensor("tab1A", [c.SPLIT, c.ROWF], F32)
    tab1B = nc.dram_tensor("tab1B", [NPAD - c.SPLIT, c.ROWF], F32)
    tab2loc = nc.dram_tensor("tab2loc", [c.NPC, c.ROWF], F32)
    tab2 = nc.dram_tensor("tab2", [c.N, c.ROWF], F32,
                          addr_space="Shared" if c.cores > 4 else "Local")

    with tile.TileContext(nc) as tc:
        with (
            tc.tile_pool(name="const", bufs=1) as constp,
            tc.tile_pool(name="slab", bufs=3) as slabp,
            tc.tile_pool(name="pkg", bufs=2) as pkgp,
            tc.tile_pool(name="eb", bufs=1) as ebp,
            tc.tile_pool(name="ertab", bufs=1) as ertabp,
            tc.tile_pool(name="gath", bufs=4) as gathp,
            tc.tile_pool(name="onehot", bufs=3) as onehotp,
            tc.tile_pool(name="sel", bufs=3) as selp,
            tc.tile_pool(name="rhsm", bufs=2) as rhsmp,
            tc.tile_pool(name="small", bufs=3) as smallp,
            tc.tile_pool(name="lt2", bufs=3) as lt2p,
            tc.tile_pool(name="outp", bufs=2) as outp,
            tc.tile_pool(name="psA", bufs=4, space="PSUM") as psA,
            tc.tile_pool(name="psB", bufs=2, space="PSUM") as psB,
            tc.tile_pool(name="psE", bufs=1, space="PSUM") as psE,
            tc.tile_pool(name="psT", bufs=1, space="PSUM") as psT,
        ):
            # ---------------- constants
            iota = constp.tile([P, P], BF16, tag="iota")
            nc.gpsimd.iota(iota[:], [[1, P]], channel_multiplier=0,
                           allow_small_or_imprecise_dtypes=True)
            iota_kp = constp.tile([P, P, K], BF16, tag="iota_kp")
            nc.gpsimd.iota(iota_kp[:], [[1, P], [0, K]], channel_multiplier=0,
                           allow_small_or_imprecise_dtypes=True)
            from concourse.masks import make_identity
            ident = constp.tile([P, P], BF16, tag="ident")
            make_identity(nc, ident[:])

            def load_const(name, param, shape, dt):
                t = constp.tile(shape, dt, tag=name, name=name)
                nc.sync.dma_start(out=t[:], in_=param[:, :])
                return t

            srcA_sb = load_const("srcA_sb", srcA16, [P, c.NT * KA * 8], I16)
            srcB_sb = load_const("srcB_sb", srcB16, [P, c.NT * KB * 8], I16)
            dstpos_sb = load_const("dstpos_sb", dstpos, [P, c.NT * K], BF16)
            erbA_sb = load_const("erbA_sb", erbA16, [P, ERR // 16], I16)
            erbB_sb = load_const("erbB_sb", erbB16, [P, ERR // 16], I16)
            mask_sb = load_const("mask_sb", maskAB, [P, 2], F32)

            rhsW = [[constp.tile([P, D2H], BF16,
                                 tag=f"rhsW{l}_{kb}", name=f"rhsW{l}_{kb}")
                     for kb in range(2)] for l in range(2)]
            for l, rt in enumerate([rhs1, rhs2]):
                for kb in range(2):
                    nc.sync.dma_start(out=rhsW[l][kb][:],
                                      in_=rt[kb * P: (kb + 1) * P, :])
            b_bc = [constp.tile([P, c.D], F32, tag=f"bbc{l}", name=f"bbc{l}")
                    for l in range(2)]
            for l, bt in enumerate([b1row, b2row]):
                nc.sync.dma_start(out=b_bc[l][:],
                                  in_=bt[0:1, :].to_broadcast([P, c.D]))

            # Pre-touch consts on compute engines so first uses don't carry
            # extra sync waits on deep pipelines.
            warm = constp.tile([P, 4], F32, tag="warm")
            warmb = warm[:].bitcast(BF16)
            nc.vector.tensor_copy(out=warm[:, 0:1], in_=dstpos_sb[:, 0:1])
            nc.vector.tensor_copy(out=warmb[:, 2:3], in_=iota[:, 0:1])
            nc.vector.tensor_copy(out=warm[:, 1:2], in_=mask_sb[:, 0:1])

            er_tab = [ertabp.tile([P, c.NT, c.H], BF16, tag=f"ertab{l}",
                                  name=f"ertab{l}")
                      for l in range(2)]

            # ---------------- phase A layer 1 (replicated, batched groups)
            def phase_a1():
                ngrp = math.ceil(c.NTA / c.G1)
                for grp in range(ngrp):
                    t0 = grp * c.G1
                    gt = min(c.G1, c.NTA - t0)
                    cols = min(gt * P, c.N - t0 * P)
                    slab = slabp.tile([P, 2, c.G1 * P], BF16, tag="slab")
                    nc.scalar.dma_start(
                        out=slab[:, :, :cols],
                        in_=xT[:, t0 * P: t0 * P + cols]
                        .rearrange("(kb p) n -> p kb n", p=P))
                    pkg = pkgp.tile([P, c.G1, c.ROWF], F32, tag="pkg")
                    pkgb = pkg[:].bitcast(BF16)
                    for j in range(gt):
                        m = min(P, c.N - (t0 + j) * P)
                        ps = psA.tile([P, D2H], F32, tag="psA")
                        for kb in range(2):
                            nc.tensor.matmul(
                                out=ps[:m, :],
                                lhsT=slab[:, kb, j * P: j * P + m],
                                rhs=rhsW[0][kb][:],
                                start=(kb == 0), stop=(kb == 1))
                        if j % 2 == 0:
                            nc.scalar.copy(out=pkgb[:m, j, : c.D],
                                           in_=ps[:m, : c.D])
                            nc.vector.tensor_copy(
                                out=pkg[:m, j, c.D // 2: WC],
                                in_=ps[:m, c.D: D2H])
                        else:
                            nc.vector.tensor_copy(out=pkgb[:m, j, : c.D],
                                                  in_=ps[:m, : c.D])
                            nc.scalar.copy(
                                out=pkg[:m, j, c.D // 2: WC],
                                in_=ps[:m, c.D: D2H])
                    r0 = t0 * P
                    rend = (t0 + gt) * P
                    pieces = []
                    for j in range(gt):
                        a = r0 + j * P
                        for dstT, lo_r, hi_r, base in (
                                (tab1A, 0, c.SPLIT, 0),
                                (tab1B, c.SPLIT, NPAD, c.SPLIT)):
                            s = max(a, lo_r)
                            e = min(a + P, hi_r)
                            if s < e:
                                pieces.append([dstT, s - base, j, s - a, e - s])
                    runs = []
                    for pc in pieces:
                        if (runs and pc[3] == 0 and pc[4] == P
                                and runs[-1][0] is pc[0]
                                and runs[-1][4] == P and runs[-1][3] == 0
                                and runs[-1][1] + runs[-1][5] * P == pc[1]
                                and runs[-1][2] + runs[-1][5] == pc[2]):
                            runs[-1][5] += 1
                        else:
                            runs.append(pc + [1 if pc[4] == P and pc[3] == 0
                                              else 0])
                    for dstT, drow0, j0, p0, n, nj in runs:
                        if nj:
                            nc.sync.dma_start(
                                out=dstT[drow0: drow0 + nj * P, :WC]
                                .rearrange("(j p) w -> p j w", p=P),
                                in_=pkg[:, j0: j0 + nj, :WC])
                        else:
                            nc.sync.dma_start(
                                out=dstT[drow0: drow0 + n, :WC],
                                in_=pkg[p0: p0 + n, j0, :WC])

            # ---------------- er_tab build for layer 1 (2 gathers + merge)
            def build_er1():
                start = c.ROWF - 64
                eroff = (c.D // 2 + c.H) - start
                half_nt = (c.NT + 1) // 2
                for piece in range(2):
                    tlo = piece * half_nt
                    tn = min(half_nt, c.NT - tlo)
                    ebs = []
                    for half, (tabh, idx_sb) in enumerate([
                            (tab1A, erbA_sb),
                            (tab1B, erbB_sb)]):
                        eb = ebp.tile([P, half_nt, 64], F32,
                                      tag=f"eb{half}", name=f"eb{half}")
                        nc.gpsimd.dma_gather(
                            out_ap=eb[:, :tn, :],
                            in_ap=tabh[:, start: start + 64],
                            idxs_ap=idx_sb[:, tlo * 8: (tlo + tn) * 8],
                            num_idxs=tn * P, num_idxs_reg=tn * P,
                            elem_size=64, elem_step=c.ROWF,
                            single_packet=False, queue_num=half)
                        ebs.append(eb)
                    tmp = smallp.tile([P, half_nt, c.H], F32, tag="ermerge")
                    nc.vector.tensor_scalar(
                        out=tmp[:, :tn], in0=ebs[0][:, :tn, eroff:eroff + c.H],
                        scalar1=mask_sb[:, 0:1], scalar2=None,
                        op0=mybir.AluOpType.mult)
                    nc.vector.scalar_tensor_tensor(
                        out=er_tab[0][:, tlo: tlo + tn],
                        in0=ebs[1][:, :tn, eroff:eroff + c.H],
                        scalar=mask_sb[:, 1:2], in1=tmp[:, :tn],
                        op0=mybir.AluOpType.mult, op1=mybir.AluOpType.add)

            # ---------------- phase B (layer = 0 or 1)
            def phase_b(layer):
                pass
                ngrp = math.ceil(c.NT / c.GB)
                for grp in range(ngrp):
                    t0 = grp * c.GB
                    gt = min(c.GB, c.NT - t0)
                    if layer == 0:
                        pkg2 = pkgp.tile([P, c.GB, c.ROWF], F32, tag="pkg2")
                        pkg2b = pkg2[:].bitcast(BF16)
                    else:
                        o2g = outp.tile([P, c.GB, c.D], F32, tag="o2g")
                    for j in range(gt):
                        t = t0 + j
                        g = gathp.tile([P, K, c.ROWF], F32, tag="gath")
                        nc.gpsimd.dma_gather(
                            out_ap=g[:, 0:KA, :],
                            in_ap=(tab1A if layer == 0 else tab2)[0: c.SPLIT, :],
                            idxs_ap=srcA_sb[:, t * KA * 8:(t + 1) * KA * 8],
                            num_idxs=KA * P, num_idxs_reg=KA * P,
                            elem_size=c.ROWF, single_packet=KA * P <= 1024)
                        nc.gpsimd.dma_gather(
                            out_ap=g[:, KA:K, :],
                            in_ap=(tab1B[0: c.N - c.SPLIT, :] if layer == 0
                                   else tab2[c.SPLIT: c.N, :]),
                            idxs_ap=srcB_sb[:, t * KB * 8:(t + 1) * KB * 8],
                            num_idxs=KB * P, num_idxs_reg=KB * P,
                            elem_size=c.ROWF, single_packet=KB * P <= 1024,
                            queue_num=1)
                        gb = g[:].bitcast(BF16)

                        # transposed one-hots for this tile (host data)
                        M_all = onehotp.tile([P, K, P], FP8, tag="M")
                        nc.scalar.dma_start(
                            out=M_all[:],
                            in_=Mhot[:, t * K * P: (t + 1) * K * P]
                            .rearrange("p (k q) -> p k q", k=K))

                        # er per slot: K tiny matmuls M_all[:, ck, :]^T @ er_tab
                        erp = psE.tile([P, K * c.H], F32, tag="psE")
                        for ck in range(K):
                            nc.tensor.matmul(
                                out=erp[:, ck * c.H:(ck + 1) * c.H],
                                lhsT=M_all[:, ck, :],
                                rhs=er_tab[layer][:, t, :],
                                start=True, stop=True)

                        # e = leakyrelu(el + er); exp
                        ea = smallp.tile([P, K, c.H], F32, tag="eadd")
                        nc.vector.tensor_tensor(
                            out=ea[:], in0=g[:, :, c.D // 2: c.D // 2 + c.H],
                            in1=erp[:].rearrange("p (k h) -> p k h", h=c.H),
                            op=mybir.AluOpType.add)
                        lr = smallp.tile([P, K, c.H], F32, tag="lrout")
                        nc.vector.scalar_tensor_tensor(
                            out=lr[:], in0=ea[:], scalar=c.NEG, in1=ea[:],
                            op0=mybir.AluOpType.mult, op1=mybir.AluOpType.max)
                        rm = rhsmp.tile([P, K, DH], BF16, tag="rhsm")
                        nc.scalar.activation(
                            out=rm[:, :, c.D: DH], in_=lr[:],
                            func=mybir.ActivationFunctionType.Exp)
                        expb = rm[:, :, c.D: DH].unsqueeze(2).to_broadcast(
                            [P, K, c.HD, c.H])
                        feat4 = gb[:, :, : c.D].rearrange(
                            "p k (hd h) -> p k hd h", h=c.H)
                        out4 = rm[:, :, : c.D].rearrange(
                            "p k (hd h) -> p k hd h", h=c.H)
                        nc.vector.tensor_tensor(out=out4, in0=feat4, in1=expb,
                                                op=mybir.AluOpType.mult)

                        # segment sum via one-hot matmuls; all K S-chunks
                        # generated in one 2x-eligible DVE op (layout [P,P,K])
                        S_allT = selp.tile([P, P, K], BF16, tag="S_allT")
                        nc.vector.tensor_tensor(
                            out=S_allT[:],
                            in0=iota_kp[:],
                            in1=dstpos_sb[:, t * K:(t + 1) * K]
                            .unsqueeze(1).to_broadcast([P, P, K]),
                            op=mybir.AluOpType.is_equal)
                        ps = psB.tile([P, DH], F32, tag="psB")
                        for ck in range(K):
                            nc.tensor.matmul(out=ps[:], lhsT=S_allT[:, :, ck],
                                             rhs=rm[:, ck, :],
                                             start=(ck == 0), stop=(ck == K - 1))
                        den = smallp.tile([P, c.H], F32, tag="den")
                        nc.vector.tensor_scalar_max(den[:], ps[:, c.D: DH], 1e-30)
                        rcp = smallp.tile([P, c.H], F32, tag="rcp")
                        nc.vector.reciprocal(rcp[:], den[:])
                        o1 = outp.tile([P, c.D], F32, tag="o1")
                        rcpb = rcp[:].unsqueeze(1).to_broadcast([P, c.HD, c.H])
                        ps4 = ps[:, : c.D].rearrange("p (hd h) -> p hd h", h=c.H)
                        o14 = o1[:].rearrange("p (hd h) -> p hd h", h=c.H)
                        nc.vector.tensor_tensor(out=o14, in0=ps4, in1=rcpb,
                                                op=mybir.AluOpType.mult)
                        nc.vector.tensor_tensor(out=o1[:], in0=o1[:],
                                                in1=b_bc[layer][:],
                                                op=mybir.AluOpType.add)
                        if layer == 0:
                            # h for this tile -> layer-2 phase A (sharded)
                            hb = outp.tile([P, c.D], BF16, tag="hb")
                            nc.vector.tensor_scalar_max(hb[:], o1[:], 0.0)
                            ps2 = psA.tile([P, D2H], F32, tag="psA")
                            for kb in range(2):
                                pst = psT.tile([P, P], BF16, tag="psT")
                                nc.tensor.transpose(
                                    out=pst[:],
                                    in_=hb[:, kb * P: (kb + 1) * P],
                                    identity=ident[:])
                                lt2 = lt2p.tile([P, P], BF16, tag="lt2")
                                nc.scalar.copy(out=lt2[:], in_=pst[:])
                                nc.tensor.matmul(
                                    out=ps2[:], lhsT=lt2[:],
                                    rhs=rhsW[1][kb][:],
                                    start=(kb == 0), stop=(kb == 1))
                            nc.scalar.copy(out=pkg2b[:, j, : c.D],
                                           in_=ps2[:, : c.D])
                            nc.scalar.copy(
                                out=pkg2[:, j, c.D // 2: c.D // 2 + c.H],
                                in_=ps2[:, c.D: c.D + c.H])
                            nc.scalar.copy(
                                out=er_tab[1][:, t, :],
                                in_=ps2[:, c.D + c.H: D2H])
                        else:
                            nc.vector.tensor_scalar_max(o2g[:, j, :], o1[:], 0.0)
                    # group epilogue
                    if layer == 0:
                        rows = min(gt * P, c.NPC - t0 * P)
                        full = rows // P
                        WEL = c.D // 2 + c.H
                        if full:
                            nc.sync.dma_start(
                                out=tab2loc[t0 * P: t0 * P + full * P, :WEL]
                                .rearrange("(j p) w -> p j w", p=P),
                                in_=pkg2[:, :full, :WEL])
                        tail = rows - full * P
                        if tail:
                            nc.sync.dma_start(
                                out=tab2loc[t0 * P + full * P:
                                            t0 * P + full * P + tail, :WEL],
                                in_=pkg2[:tail, full, :WEL])
                    else:
                        nc.sync.dma_start(
                            out=out_ext[t0 * P: (t0 + gt) * P, :]
                            .rearrange("(j p) d -> p j d", p=P),
                            in_=o2g[:, :gt, :])

            for _rep in range(reps):
                phase_a1()
                build_er1()
                phase_b(0)
                nc.gpsimd.collective_compute(
                    "AllGather",
                    mybir.AluOpType.bypass,
                    replica_groups=[list(range(c.cores))],
                    ins=[tab2loc[:]],
                    outs=[tab2[:]],
                )
                phase_b(1)

    nc.compile()
    return nc


# ---------------------------------------------------------------- reference

def ref_np(inputs, cfg):
    c = cfg
    x = np.asarray(inputs["data"], np.float64)
    src = np.asarray(inputs["src"]).astype(np.int64)
    dst = np.asarray(inputs["dst"]).astype(np.int64)

    def layer(x, W, al, ar, b):
        N = x.shape[0]
        feat = (x @ np.asarray(W, np.float64)).reshape(N, c.H, c.HD)
        el = np.einsum("nhd,hd->nh", feat, np.asarray(al, np.float64))
        er = np.einsum("nhd,hd->nh", feat, np.asarray(ar, np.float64))
        e = el[src] + er[dst]
        e = np.where(e > 0, e, c.NEG * e)
        m = np.full((N, c.H), -np.inf)
        np.maximum.at(m, dst, e)
        a = np.exp(e - m[dst])
        den = np.zeros((N, c.H))
        np.add.at(den, dst, a)
        alpha = a / den[dst]
        msg = feat[src] * alpha[:, :, None]
        out = np.zeros((N, c.H, c.HD))
        np.add.at(out, dst, msg)
        out = out + np.asarray(b, np.float64).reshape(1, c.H, c.HD)
        return np.maximum(out, 0).reshape(N, c.D)

    h = layer(x, inputs["W1"], inputs["al1"], inputs["ar1"], inputs["b1"])
    h = layer(h, inputs["W2"], inputs["al2"], inputs["ar2"], inputs["b2"])
    return h


# ---------------------------------------------------------------- entry point

_BUILD_CACHE = {}


def kernel(**inputs) -> np.ndarray:
    """Full-input GAT kernel: shards internally across 8 NeuronCores."""
    from concourse.bass_utils import run_bass_kernel_spmd

    cfg = make_cfg(N=50000, E=800000, D=256, H=8, cores=8)
    in_maps, meta = prep_all(inputs, cfg)
    key = (meta.KA, meta.KB)
    if key not in _BUILD_CACHE:
        _BUILD_CACHE[key] = build_nc(cfg, meta.KA, meta.KB)
    nc = _BUILD_CACHE[key]
    res = run_bass_kernel_spmd(nc, in_maps, list(range(cfg.cores)))
    results = [{"out": res.results[ci]["out"]} for ci in range(cfg.cores)]
    out = finalize(results, cfg, meta)
    return np.ascontiguousarray(out.astype(np.float32))


# revision 8
# speedup vs baseline: 1.0119x; 1.0003x over previous
"""2-layer GAT (DGL GATConv style) on 8 trn2 NeuronCores via Bass/Tile. v2

Design:
  - Edges dst-sorted on host; cores own contiguous equal node ranges
    [c*NPC, (c+1)*NPC) and all edges whose dst falls in range.
  - Layer 1 phase A (replicated): packed table row (f32 container)
    [feat(D bf16) | el(H f32) | er(H f32) | pad] = x @ [W | Wal | War],
    written to tab1 in groups of 8 tiles (one slab load + one table write
    per group). Feature cols h-innermost (packed col j <-> head j%H).
  - er for own nodes lives in SBUF (er_tab[d, t, h], d = dst position in
    tile t). Layer 1 builds it with two small gathers (tabA/tabB halves,
    host-clipped indices) + mask merge; layer 2 captures it for free from
    phase-A PSUM (the layer-2 transform is sharded, tiles align).
  - Phase B (sharded): per dst-tile, gather src rows (768 B each) with
    dma_gather. S one-hots (slot->dst, for segment-sum matmuls) are
    generated in ONE 2x-rate DVE op per tile (layout [P, P, K] keeps every
    operand innermost-packed); M one-hots (dst->slot transposes, for the
    er broadcast) are host-built fp8 data streamed from DRAM (0/1 is exact
    in fp8; half the bytes of bf16). er per slot = K tiny M^T @ er_tab
    matmuls instead of a 256 B/edge gather. exp(leakyrelu(el+er)) ->
    weighted messages -> segment sum via S matmuls into PSUM [num | den];
    normalize, bias, relu.
  - Layer 2 phase A is fused into phase B of layer 1: each B0 tile's
    output hb is PE-transposed and immediately transformed (2 matmuls) to
    its packed table row; rows go to tab2loc [NPC, ROWF]. After B0 an
    AllGather concatenates the 8 slices into the full tab2 - no hT
    roundtrip and no replicated layer-2 transform.
"""

import math
import sys
from types import SimpleNamespace

import numpy as np

sys.path.insert(0, "/opt/trn_rl_repo")

from concourse import bacc, bass, mybir, tile  # noqa: E402

F32 = mybir.dt.float32
BF16 = mybir.dt.bfloat16
FP8 = mybir.dt.float8e4
I32 = mybir.dt.int32
I16 = mybir.dt.int16

P = 128


def make_cfg(N=50000, E=800000, D=256, H=8, cores=8, split=None):
    HD = D // H
    NPC = N // cores
    NT = math.ceil(NPC / P)          # dst tiles per core
    NTA = math.ceil(N / P)           # phase-A node tiles (layer 1, flat)
    ROWU = ((D + 4 * H + 127) // 128) * 128   # packed row, bf16 units (256B mult)
    if split is None:
        split = NPC * min(cores, 32767 // NPC)
        split = min(split, N)
    assert split % NPC == 0 and split <= 32767 + 1 and N - split <= 32767 + 1
    return SimpleNamespace(
        N=N, E=E, D=D, H=H, HD=HD, cores=cores, NPC=NPC, NT=NT, NTA=NTA,
        ROWU=ROWU, ROWF=ROWU // 2, SPLIT=split,
        G1=8, GB=7,
        NEG=0.2,
    )


# ---------------------------------------------------------------- host prep

def perm_h_inner(D, H):
    """perm[j] = original feature index stored at packed col j (h-innermost)."""
    HD = D // H
    j = np.arange(D)
    return (j % H) * HD + j // H


def attn_cols(W, a, H):
    D = W.shape[0]
    HD = W.shape[1] // H
    return np.stack(
        [W[:, h * HD:(h + 1) * HD] @ a[h] for h in range(H)], axis=1
    )


def wrap16(flat, reps=8):
    """[num] -> [16*reps, num//16] int16 wrapped layout (idx i at [i%16, i//16]),
    replicated across the 8 gpsimd cores."""
    num = len(flat)
    assert num % 16 == 0
    a = np.zeros((16, num // 16), dtype=np.int16)
    a[np.arange(num) % 16, np.arange(num) // 16] = flat
    return np.tile(a, (reps, 1))


def prep_edges(src, dst, cfg):
    c = cfg
    order = np.argsort(dst, kind="stable")
    src_s = src[order].astype(np.int64)
    dst_s = dst[order].astype(np.int64)

    core = dst_s // c.NPC
    loc = dst_s - core * c.NPC
    lt = loc // P
    pos = loc - lt * P
    islow = src_s < c.SPLIT

    # group = (core, tile, islow) ; edges of a group get consecutive slots
    g = (core * c.NT + lt) * 2 + (1 - islow)   # low first
    order2 = np.argsort(g, kind="stable")
    src_s, dst_s, core, lt, pos, islow, g = (
        a[order2] for a in (src_s, dst_s, core, lt, pos, islow, g))

    uniq, starts = np.unique(g, return_index=True)
    start_of = np.zeros(c.cores * c.NT * 2, dtype=np.int64)
    start_of[uniq] = starts
    rank = np.arange(len(dst_s)) - start_of[g]

    counts = np.zeros(c.cores * c.NT * 2, dtype=np.int64)
    np.add.at(counts, g, 1)
    KA = int(math.ceil(counts[0::2].max() / P))
    KB = int(math.ceil(counts[1::2].max() / P)) if counts[1::2].max() > 0 else 0
    K = KA + KB

    # slot within tile: low edges fill chunks [0,KA), high fill [KA,K)
    chunk = rank // P + np.where(islow, 0, KA)
    part = rank % P

    srcA = np.zeros((c.cores, c.NT * KA * P), dtype=np.int64)
    srcB = np.zeros((c.cores, c.NT * KB * P), dtype=np.int64) if KB else None
    dstpos = np.full((c.cores, P, c.NT * K), 255.0, dtype=np.float32)
    dstposT = np.full((c.cores, c.NT * K * P), 255, dtype=np.int16)

    iA = lt * (KA * P) + (chunk * P + part)            # valid where islow
    iB = lt * (KB * P) + ((chunk - KA) * P + part) if KB else None
    low = islow
    srcA[core[low], iA[low]] = src_s[low]
    if KB:
        hi = ~islow
        srcB[core[hi], iB[hi]] = src_s[hi] - c.SPLIT
    dstpos[core, part, lt * K + chunk] = pos
    dstposT[core, (lt * K + chunk) * P + part] = pos

    # host-built transposed one-hots M[d, (t, ck, p)] = (dstposT == d).
    # fp8e4m3 is exact for 0/1 and halves the stream bytes.
    import ml_dtypes
    Mhot = (dstposT[:, None, :] == np.arange(P, dtype=np.int16)[None, :, None]
            ).astype(ml_dtypes.float8_e4m3fn)

    srcA16 = np.stack([wrap16(srcA[ci]) for ci in range(c.cores)])
    srcB16 = (np.stack([wrap16(srcB[ci]) for ci in range(c.cores)])
              if KB else np.zeros((c.cores, P, 0), np.int16))
    return srcA16, srcB16, dstpos, Mhot, KA, KB


def prep_all(inputs, cfg):
    c = cfg
    perm = perm_h_inner(c.D, c.H)
    x = np.asarray(inputs["data"], np.float32)
    src = np.asarray(inputs["src"]).astype(np.int64)
    dst = np.asarray(inputs["dst"]).astype(np.int64)

    def rhs_for(W, al, ar, permute_rows):
        W = np.asarray(W, np.float64)
        Wal = attn_cols(W, np.asarray(al, np.float64), c.H)
        War = attn_cols(W, np.asarray(ar, np.float64), c.H)
        Wp = W[:, perm]
        if permute_rows:
            Wp, Wal, War = Wp[perm], Wal[perm], War[perm]
        return to_bf16(np.concatenate([Wp, Wal, War], axis=1))

    rhs1 = rhs_for(inputs["W1"], inputs["al1"], inputs["ar1"], False)
    rhs2 = rhs_for(inputs["W2"], inputs["al2"], inputs["ar2"], True)
    b1 = np.asarray(inputs["b1"], np.float32)[perm].reshape(1, c.D)
    b2 = np.asarray(inputs["b2"], np.float32)[perm].reshape(1, c.D)
    xT = to_bf16(x.T.copy())

    srcA16, srcB16, dstpos, Mhot, KA, KB = prep_edges(src, dst, c)

    # er build indices (layer 1): row i (i in [0, NT*128)) <- own node NPC*ci+i
    erbA, erbB, masks = [], [], []
    for ci in range(c.cores):
        base = ci * c.NPC
        rows = np.arange(c.NT * P)
        ra = np.where(base < c.SPLIT, base + rows, 0)
        ra = np.clip(ra, 0, c.SPLIT - 1)
        rb = np.where(base >= c.SPLIT, base - c.SPLIT + rows, 0)
        rb = np.clip(rb, 0, max(c.N - c.SPLIT - 1, 0))
        erbA.append(wrap16(ra))
        erbB.append(wrap16(rb))
        mA = 1.0 if base < c.SPLIT else 0.0
        m = np.zeros((P, 2), np.float32)
        m[:, 0] = mA
        m[:, 1] = 1.0 - mA
        masks.append(m)

    meta = SimpleNamespace(perm=perm, KA=KA, KB=KB, K=KA + KB)
    in_maps = []
    for ci in range(c.cores):
        in_maps.append({
            "xT": xT, "rhs1": rhs1, "rhs2": rhs2, "b1row": b1, "b2row": b2,
            "srcA16": srcA16[ci], "srcB16": srcB16[ci],
            "dstpos": to_bf16(dstpos[ci]),
            "Mhot": Mhot[ci],
            "erbA16": erbA[ci], "erbB16": erbB[ci],
            "maskAB": masks[ci],
        })
    return in_maps, meta


def to_bf16(a):
    import ml_dtypes
    return np.asarray(a).astype(ml_dtypes.bfloat16)


def finalize(results, cfg, meta):
    c = cfg
    parts = [results[ci]["out"][: c.NPC] for ci in range(c.cores)]
    out_p = np.concatenate(parts, axis=0)
    out = np.empty_like(out_p)
    out[:, meta.perm] = out_p
    return out


# ---------------------------------------------------------------- kernel

def build_nc(cfg, KA, KB, debug=False, reps=1):
    c = cfg
    K = KA + KB
    ERR = c.NT * P                   # er rows per core (padded)
    NPAD = c.NTA * P                 # padded table rows (layer 1)
    DH = c.D + c.H                   # seg-matmul rhs cols (msg + den)
    D2H = c.D + 2 * c.H              # phase-A rhs cols
    WC = c.D // 2 + 2 * c.H          # written f32 cols of a packed row

    nc = bacc.Bacc("TRN2", target_bir_lowering=False, debug=debug,
                   num_devices=c.cores, num_swdge_queues=2)

    xT = nc.declare_dram_parameter("xT", [c.D, c.N], BF16, isOutput=False)
    rhs1 = nc.declare_dram_parameter("rhs1", [c.D, D2H], BF16, isOutput=False)
    rhs2 = nc.declare_dram_parameter("rhs2", [c.D, D2H], BF16, isOutput=False)
    b1row = nc.declare_dram_parameter("b1row", [1, c.D], F32, isOutput=False)
    b2row = nc.declare_dram_parameter("b2row", [1, c.D], F32, isOutput=False)
    srcA16 = nc.declare_dram_parameter("srcA16", [P, c.NT * KA * 8], I16, isOutput=False)
    srcB16 = nc.declare_dram_parameter("srcB16", [P, c.NT * KB * 8], I16, isOutput=False)
    dstpos = nc.declare_dram_parameter("dstpos", [P, c.NT * K], BF16, isOutput=False)
    Mhot = nc.declare_dram_parameter("Mhot", [P, c.NT * K * P], FP8, isOutput=False)
    erbA16 = nc.declare_dram_parameter("erbA16", [P, ERR // 16], I16, isOutput=False)
    erbB16 = nc.declare_dram_parameter("erbB16", [P, ERR // 16], I16, isOutput=False)
    maskAB = nc.declare_dram_parameter("maskAB", [P, 2], F32, isOutput=False)
    out_ext = nc.declare_dram_parameter("out", [c.NT * P, c.D], F32, isOutput=True)

    tab1A = nc.dram_tensor("tab1A", [c.SPLIT, c.ROWF], F32)
    tab1B = nc.dram_tensor("tab1B", [NPAD - c.SPLIT, c.ROWF], F32)
    tab2loc = nc.dram_tensor("tab2loc", [c.NPC, c.ROWF], F32)
    tab2 = nc.dram_tensor("tab2", [c.N, c.ROWF], F32,
                          addr_space="Shared" if c.cores > 4 else "Local")

    with tile.TileContext(nc) as tc:
        with (
            tc.tile_pool(name="const", bufs=1) as constp,
            tc.tile_pool(name="slab", bufs=4) as slabp,
            tc.tile_pool(name="pkg", bufs=2) as pkgp,
            tc.tile_pool(name="eb", bufs=1) as ebp,
            tc.tile_pool(name="ertab", bufs=1) as ertabp,
            tc.tile_pool(name="gath", bufs=4) as gathp,
            tc.tile_pool(name="onehot", bufs=3) as onehotp,
            tc.tile_pool(name="sel", bufs=3) as selp,
            tc.tile_pool(name="rhsm", bufs=2) as rhsmp,
            tc.tile_pool(name="small", bufs=3) as smallp,
            tc.tile_pool(name="lt2", bufs=3) as lt2p,
            tc.tile_pool(name="outp", bufs=2) as outp,
            tc.tile_pool(name="psA", bufs=4, space="PSUM") as psA,
            tc.tile_pool(name="psB", bufs=2, space="PSUM") as psB,
            tc.tile_pool(name="psE", bufs=1, space="PSUM") as psE,
            tc.tile_pool(name="psT", bufs=1, space="PSUM") as psT,
        ):
            # ---------------- constants
            iota = constp.tile([P, P], BF16, tag="iota")
            nc.gpsimd.iota(iota[:], [[1, P]], channel_multiplier=0,
                           allow_small_or_imprecise_dtypes=True)
            iota_kp = constp.tile([P, P, K], BF16, tag="iota_kp")
            nc.gpsimd.iota(iota_kp[:], [[1, P], [0, K]], channel_multiplier=0,
                           allow_small_or_imprecise_dtypes=True)
            from concourse.masks import make_identity
            ident = constp.tile([P, P], BF16, tag="ident")
            make_identity(nc, ident[:])

            def load_const(name, param, shape, dt):
                t = constp.tile(shape, dt, tag=name, name=name)
                nc.sync.dma_start(out=t[:], in_=param[:, :])
                return t

            srcA_sb = load_const("srcA_sb", srcA16, [P, c.NT * KA * 8], I16)
            srcB_sb = load_const("srcB_sb", srcB16, [P, c.NT * KB * 8], I16)
            dstpos_sb = load_const("dstpos_sb", dstpos, [P, c.NT * K], BF16)
            erbA_sb = load_const("erbA_sb", erbA16, [P, ERR // 16], I16)
            erbB_sb = load_const("erbB_sb", erbB16, [P, ERR // 16], I16)
            mask_sb = load_const("mask_sb", maskAB, [P, 2], F32)

            rhsW = [[constp.tile([P, D2H], BF16,
                                 tag=f"rhsW{l}_{kb}", name=f"rhsW{l}_{kb}")
                     for kb in range(2)] for l in range(2)]
            for l, rt in enumerate([rhs1, rhs2]):
                for kb in range(2):
                    nc.sync.dma_start(out=rhsW[l][kb][:],
                                      in_=rt[kb * P: (kb + 1) * P, :])
            b_bc = [constp.tile([P, c.D], F32, tag=f"bbc{l}", name=f"bbc{l}")
                    for l in range(2)]
            for l, bt in enumerate([b1row, b2row]):
                nc.sync.dma_start(out=b_bc[l][:],
                                  in_=bt[0:1, :].to_broadcast([P, c.D]))

            # Pre-touch consts on compute engines so first uses don't carry
            # extra sync waits on deep pipelines.
            warm = constp.tile([P, 4], F32, tag="warm")
            warmb = warm[:].bitcast(BF16)
            nc.vector.tensor_copy(out=warm[:, 0:1], in_=dstpos_sb[:, 0:1])
            nc.vector.tensor_copy(out=warmb[:, 2:3], in_=iota[:, 0:1])
            nc.vector.tensor_copy(out=warm[:, 1:2], in_=mask_sb[:, 0:1])

            er_tab = [ertabp.tile([P, c.NT, c.H], BF16, tag=f"ertab{l}",
                                  name=f"ertab{l}")
                      for l in range(2)]

            # ---------------- phase A layer 1 (replicated, batched groups)
            def phase_a1():
                ngrp = math.ceil(c.NTA / c.G1)
                for grp in range(ngrp):
                    t0 = grp * c.G1
                    gt = min(c.G1, c.NTA - t0)
                    cols = min(gt * P, c.N - t0 * P)
                    slab = slabp.tile([P, 2, c.G1 * P], BF16, tag="slab")
                    nc.scalar.dma_start(
                        out=slab[:, :, :cols],
                        in_=xT[:, t0 * P: t0 * P + cols]
                        .rearrange("(kb p) n -> p kb n", p=P))
                    pkg = pkgp.tile([P, c.G1, c.ROWF], F32, tag="pkg")
                    pkgb = pkg[:].bitcast(BF16)
                    for j in range(gt):
                        m = min(P, c.N - (t0 + j) * P)
                        ps = psA.tile([P, D2H], F32, tag="psA")
                        for kb in range(2):
                            nc.tensor.matmul(
                                out=ps[:m, :],
                                lhsT=slab[:, kb, j * P: j * P + m],
                                rhs=rhsW[0][kb][:],
                                start=(kb == 0), stop=(kb == 1))
                        if j % 2 == 0:
                            nc.scalar.copy(out=pkgb[:m, j, : c.D],
                                           in_=ps[:m, : c.D])
                            nc.vector.tensor_copy(
                                out=pkg[:m, j, c.D // 2: WC],
                                in_=ps[:m, c.D: D2H])
                        else:
                            nc.vector.tensor_copy(out=pkgb[:m, j, : c.D],
                                                  in_=ps[:m, : c.D])
                            nc.scalar.copy(
                                out=pkg[:m, j, c.D // 2: WC],
                                in_=ps[:m, c.D: D2H])
                    r0 = t0 * P
                    rend = (t0 + gt) * P
                    pieces = []
                    for j in range(gt):
                        a = r0 + j * P
                        for dstT, lo_r, hi_r, base in (
                                (tab1A, 0, c.SPLIT, 0),
                                (tab1B, c.SPLIT, NPAD, c.SPLIT)):
                            s = max(a, lo_r)
                            e = min(a + P, hi_r)
                            if s < e:
                                pieces.append([dstT, s - base, j, s - a, e - s])
                    runs = []
                    for pc in pieces:
                        if (runs and pc[3] == 0 and pc[4] == P
                                and runs[-1][0] is pc[0]
                                and runs[-1][4] == P and runs[-1][3] == 0
                                and runs[-1][1] + runs[-1][5] * P == pc[1]
                                and runs[-1][2] + runs[-1][5] == pc[2]):
                            runs[-1][5] += 1
                        else:
                            runs.append(pc + [1 if pc[4] == P and pc[3] == 0
                                              else 0])
                    for dstT, drow0, j0, p0, n, nj in runs:
                        if nj:
                            nc.sync.dma_start(
                                out=dstT[drow0: drow0 + nj * P, :WC]
                                .rearrange("(j p) w -> p j w", p=P),
                                in_=pkg[:, j0: j0 + nj, :WC])
                        else:
                            nc.sync.dma_start(
                                out=dstT[drow0: drow0 + n, :WC],
                                in_=pkg[p0: p0 + n, j0, :WC])

            # ---------------- er_tab build for layer 1 (2 gathers + merge)
            def build_er1():
                start = c.ROWF - 64
                eroff = (c.D // 2 + c.H) - start
                half_nt = (c.NT + 1) // 2
                for piece in range(2):
                    tlo = piece * half_nt
                    tn = min(half_nt, c.NT - tlo)
                    ebs = []
                    for half, (tabh, idx_sb) in enumerate([
                            (tab1A, erbA_sb),
                            (tab1B, erbB_sb)]):
                        eb = ebp.tile([P, half_nt, 64], F32,
                                      tag=f"eb{half}", name=f"eb{half}")
                        nc.gpsimd.dma_gather(
                            out_ap=eb[:, :tn, :],
                            in_ap=tabh[:, start: start + 64],
                            idxs_ap=idx_sb[:, tlo * 8: (tlo + tn) * 8],
                            num_idxs=tn * P, num_idxs_reg=tn * P,
                            elem_size=64, elem_step=c.ROWF,
                            single_packet=False, queue_num=half)
                        ebs.append(eb)
                    tmp = smallp.tile([P, half_nt, c.H], F32, tag="ermerge")
                    nc.vector.tensor_scalar(
                        out=tmp[:, :tn], in0=ebs[0][:, :tn, eroff:eroff + c.H],
                        scalar1=mask_sb[:, 0:1], scalar2=None,
                        op0=mybir.AluOpType.mult)
                    nc.vector.scalar_tensor_tensor(
                        out=er_tab[0][:, tlo: tlo + tn],
                        in0=ebs[1][:, :tn, eroff:eroff + c.H],
                        scalar=mask_sb[:, 1:2], in1=tmp[:, :tn],
                        op0=mybir.AluOpType.mult, op1=mybir.AluOpType.add)

            # ---------------- phase B (layer = 0 or 1)
            def phase_b(layer):
                pass
                ngrp = math.ceil(c.NT / c.GB)
                for grp in range(ngrp):
                    t0 = grp * c.GB
                    gt = min(c.GB, c.NT - t0)
                    if layer == 0:
                        pkg2 = pkgp.tile([P, c.GB, c.ROWF], F32, tag="pkg2")
                        pkg2b = pkg2[:].bitcast(BF16)
                    else:
                        o2g = outp.tile([P, c.GB, c.D], F32, tag="o2g")
                    for j in range(gt):
                        t = t0 + j
                        g = gathp.tile([P, K, c.ROWF], F32, tag="gath")
                        nc.gpsimd.dma_gather(
                            out_ap=g[:, 0:KA, :],
                            in_ap=(tab1A if layer == 0 else tab2)[0: c.SPLIT, :],
                            idxs_ap=srcA_sb[:, t * KA * 8:(t + 1) * KA * 8],
                            num_idxs=KA * P, num_idxs_reg=KA * P,
                            elem_size=c.ROWF, single_packet=KA * P <= 1024)
                        nc.gpsimd.dma_gather(
                            out_ap=g[:, KA:K, :],
                            in_ap=(tab1B[0: c.N - c.SPLIT, :] if layer == 0
                                   else tab2[c.SPLIT: c.N, :]),
                            idxs_ap=srcB_sb[:, t * KB * 8:(t + 1) * KB * 8],
                            num_idxs=KB * P, num_idxs_reg=KB * P,
                            elem_size=c.ROWF, single_packet=KB * P <= 1024,
                            queue_num=1)
                        gb = g[:].bitcast(BF16)

                        # transposed one-hots for this tile (host data)
                        M_all = onehotp.tile([P, K, P], FP8, tag="M")
                        nc.scalar.dma_start(
                            out=M_all[:],
                            in_=Mhot[:, t * K * P: (t + 1) * K * P]
                            .rearrange("p (k q) -> p k q", k=K))

                        # er per slot: K tiny matmuls M_all[:, ck, :]^T @ er_tab
                        erp = psE.tile([P, K * c.H], F32, tag="psE")
                        for ck in range(K):
                            nc.tensor.matmul(
                                out=erp[:, ck * c.H:(ck + 1) * c.H],
                                lhsT=M_all[:, ck, :],
                                rhs=er_tab[layer][:, t, :],
                                start=True, stop=True)

                        # e = leakyrelu(el + er); exp
                        ea = smallp.tile([P, K, c.H], F32, tag="eadd")
                        nc.vector.tensor_tensor(
                            out=ea[:], in0=g[:, :, c.D // 2: c.D // 2 + c.H],
                            in1=erp[:].rearrange("p (k h) -> p k h", h=c.H),
                            op=mybir.AluOpType.add)
                        lr = smallp.tile([P, K, c.H], F32, tag="lrout")
                        nc.vector.scalar_tensor_tensor(
                            out=lr[:], in0=ea[:], scalar=c.NEG, in1=ea[:],
                            op0=mybir.AluOpType.mult, op1=mybir.AluOpType.max)
                        rm = rhsmp.tile([P, K, DH], BF16, tag="rhsm")
                        nc.scalar.activation(
                            out=rm[:, :, c.D: DH], in_=lr[:],
                            func=mybir.ActivationFunctionType.Exp)
                        expb = rm[:, :, c.D: DH].unsqueeze(2).to_broadcast(
                            [P, K, c.HD, c.H])
                        feat4 = gb[:, :, : c.D].rearrange(
                            "p k (hd h) -> p k hd h", h=c.H)
                        out4 = rm[:, :, : c.D].rearrange(
                            "p k (hd h) -> p k hd h", h=c.H)
                        nc.vector.tensor_tensor(out=out4, in0=feat4, in1=expb,
                                                op=mybir.AluOpType.mult)

                        # segment sum via one-hot matmuls; all K S-chunks
                        # generated in one 2x-eligible DVE op (layout [P,P,K])
                        S_allT = selp.tile([P, P, K], BF16, tag="S_allT")
                        nc.vector.tensor_tensor(
                            out=S_allT[:],
                            in0=iota_kp[:],
                            in1=dstpos_sb[:, t * K:(t + 1) * K]
                            .unsqueeze(1).to_broadcast([P, P, K]),
                            op=mybir.AluOpType.is_equal)
                        ps = psB.tile([P, DH], F32, tag="psB")
                        for ck in range(K):
                            nc.tensor.matmul(out=ps[:], lhsT=S_allT[:, :, ck],
                                             rhs=rm[:, ck, :],
                                             start=(ck == 0), stop=(ck == K - 1))
                        den = smallp.tile([P, c.H], F32, tag="den")
                        nc.vector.tensor_scalar_max(den[:], ps[:, c.D: DH], 1e-30)
                        rcp = smallp.tile([P, c.H], F32, tag="rcp")
                        nc.vector.reciprocal(rcp[:], den[:])
                        o1 = outp.tile([P, c.D], F32, tag="o1")
                        rcpb = rcp[:].unsqueeze(1).to_broadcast([P, c.HD, c.H])
                        ps4 = ps[:, : c.D].rearrange("p (hd h) -> p hd h", h=c.H)
                        o14 = o1[:].rearrange("p (hd h) -> p hd h", h=c.H)
                        nc.vector.tensor_tensor(out=o14, in0=ps4, in1=rcpb,
                                                op=mybir.AluOpType.mult)
                        nc.vector.tensor_tensor(out=o1[:], in0=o1[:],
                                                in1=b_bc[layer][:],
                                                op=mybir.AluOpType.add)
                        if layer == 0:
                            # h for this tile -> layer-2 phase A (sharded)
                            hb = outp.tile([P, c.D], BF16, tag="hb")
                            nc.vector.tensor_scalar_max(hb[:], o1[:], 0.0)
                            ps2 = psA.tile([P, D2H], F32, tag="psA")
                            for kb in range(2):
                                pst = psT.tile([P, P], BF16, tag="psT")
                                nc.tensor.transpose(
                                    out=pst[:],
                                    in_=hb[:, kb * P: (kb + 1) * P],
                                    identity=ident[:])
                                lt2 = lt2p.tile([P, P], BF16, tag="lt2")
                                nc.scalar.copy(out=lt2[:], in_=pst[:])
                                nc.tensor.matmul(
                                    out=ps2[:], lhsT=lt2[:],
                                    rhs=rhsW[1][kb][:],
                                    start=(kb == 0), stop=(kb == 1))
                            nc.scalar.copy(out=pkg2b[:, j, : c.D],
                                           in_=ps2[:, : c.D])
                            nc.scalar.copy(
                                out=pkg2[:, j, c.D // 2: c.D // 2 + c.H],
                                in_=ps2[:, c.D: c.D + c.H])
                            nc.scalar.copy(
                                out=er_tab[1][:, t, :],
                                in_=ps2[:, c.D + c.H: D2H])
                        else:
                            nc.vector.tensor_scalar_max(o2g[:, j, :], o1[:], 0.0)
                    # group epilogue
                    if layer == 0:
                        rows = min(gt * P, c.NPC - t0 * P)
                        full = rows // P
                        WEL = c.D // 2 + c.H
                        if full:
                            nc.sync.dma_start(
                                out=tab2loc[t0 * P: t0 * P + full * P, :WEL]
                                .rearrange("(j p) w -> p j w", p=P),
                                in_=pkg2[:, :full, :WEL])
                        tail = rows - full * P
                        if tail:
                            nc.sync.dma_start(
                                out=tab2loc[t0 * P + full * P:
                                            t0 * P + full * P + tail, :WEL],
                                in_=pkg2[:tail, full, :WEL])
                    else:
                        nc.sync.dma_start(
                            out=out_ext[t0 * P: (t0 + gt) * P, :]
                            .rearrange("(j p) d -> p j d", p=P),
                            in_=o2g[:, :gt, :])

            for _rep in range(reps):
                phase_a1()
                build_er1()
                phase_b(0)
                nc.gpsimd.collective_compute(
                    "AllGather",
                    mybir.AluOpType.bypass,
                    replica_groups=[list(range(c.cores))],
                    ins=[tab2loc[:]],
                    outs=[tab2[:]],
                )
                phase_b(1)

    nc.compile()
    return nc


# ---------------------------------------------------------------- reference

def ref_np(inputs, cfg):
    c = cfg
    x = np.asarray(inputs["data"], np.float64)
    src = np.asarray(inputs["src"]).astype(np.int64)
    dst = np.asarray(inputs["dst"]).astype(np.int64)

    def layer(x, W, al, ar, b):
        N = x.shape[0]
        feat = (x @ np.asarray(W, np.float64)).reshape(N, c.H, c.HD)
        el = np.einsum("nhd,hd->nh", feat, np.asarray(al, np.float64))
        er = np.einsum("nhd,hd->nh", feat, np.asarray(ar, np.float64))
        e = el[src] + er[dst]
        e = np.where(e > 0, e, c.NEG * e)
        m = np.full((N, c.H), -np.inf)
        np.maximum.at(m, dst, e)
        a = np.exp(e - m[dst])
        den = np.zeros((N, c.H))
        np.add.at(den, dst, a)
        alpha = a / den[dst]
        msg = feat[src] * alpha[:, :, None]
        out = np.zeros((N, c.H, c.HD))
        np.add.at(out, dst, msg)
        out = out + np.asarray(b, np.float64).reshape(1, c.H, c.HD)
        return np.maximum(out, 0).reshape(N, c.D)

    h = layer(x, inputs["W1"], inputs["al1"], inputs["ar1"], inputs["b1"])
    h = layer(h, inputs["W2"], inputs["al2"], inputs["ar2"], inputs["b2"])
    return h


# ---------------------------------------------------------------- entry point

_BUILD_CACHE = {}


def kernel(**inputs) -> np.ndarray:
    """Full-input GAT kernel: shards internally across 8 NeuronCores."""
    from concourse.bass_utils import run_bass_kernel_spmd

    cfg = make_cfg(N=50000, E=800000, D=256, H=8, cores=8)
    in_maps, meta = prep_all(inputs, cfg)
    key = (meta.KA, meta.KB)
    if key not in _BUILD_CACHE:
        _BUILD_CACHE[key] = build_nc(cfg, meta.KA, meta.KB)
    nc = _BUILD_CACHE[key]
    res = run_bass_kernel_spmd(nc, in_maps, list(range(cfg.cores)))
    results = [{"out": res.results[ci]["out"]} for ci in range(cfg.cores)]
    out = finalize(results, cfg, meta)
    return np.ascontiguousarray(out.astype(np.float32))


# revision 9
# speedup vs baseline: 1.0140x; 1.0021x over previous
"""2-layer GAT (DGL GATConv style) on 8 trn2 NeuronCores via Bass/Tile. v2

Design:
  - Edges dst-sorted on host; cores own contiguous equal node ranges
    [c*NPC, (c+1)*NPC) and all edges whose dst falls in range.
  - Layer 1 phase A (replicated): packed table row (f32 container)
    [feat(D bf16) | el(H f32) | er(H f32) | pad] = x @ [W | Wal | War],
    written to tab1 in groups of 8 tiles (one slab load + one table write
    per group). Feature cols h-innermost (packed col j <-> head j%H).
  - er for own nodes lives in SBUF (er_tab[d, t, h], d = dst position in
    tile t). Layer 1 builds it with two small gathers (tabA/tabB halves,
    host-clipped indices) + mask merge; layer 2 captures it for free from
    phase-A PSUM (the layer-2 transform is sharded, tiles align).
  - Phase B (sharded): per dst-tile, gather src rows (768 B each) with
    dma_gather. S one-hots (slot->dst, for segment-sum matmuls) are
    generated in ONE 2x-rate DVE op per tile (layout [P, P, K] keeps every
    operand innermost-packed); M one-hots (dst->slot transposes, for the
    er broadcast) are host-built fp8 data streamed from DRAM (0/1 is exact
    in fp8; half the bytes of bf16). er per slot = K tiny M^T @ er_tab
    matmuls instead of a 256 B/edge gather. exp(leakyrelu(el+er)) ->
    weighted messages -> segment sum via S matmuls into PSUM [num | den];
    normalize, bias, relu.
  - Layer 2 phase A is fused into phase B of layer 1: each B0 tile's
    output hb is PE-transposed and immediately transformed (2 matmuls) to
    its packed table row; rows go to tab2loc [NPC, ROWF]. After B0 an
    AllGather concatenates the 8 slices into the full tab2 - no hT
    roundtrip and no replicated layer-2 transform.
"""

import math
import sys
from types import SimpleNamespace

import numpy as np

sys.path.insert(0, "/opt/trn_rl_repo")

from concourse import bacc, bass, mybir, tile  # noqa: E402

F32 = mybir.dt.float32
BF16 = mybir.dt.bfloat16
FP8 = mybir.dt.float8e4
I32 = mybir.dt.int32
I16 = mybir.dt.int16

P = 128


def make_cfg(N=50000, E=800000, D=256, H=8, cores=8, split=None):
    HD = D // H
    NPC = N // cores
    NT = math.ceil(NPC / P)          # dst tiles per core
    NTA = math.ceil(N / P)           # phase-A node tiles (layer 1, flat)
    ROWU = ((D + 4 * H + 127) // 128) * 128   # packed row, bf16 units (256B mult)
    if split is None:
        split = NPC * min(cores, 32767 // NPC)
        split = min(split, N)
    assert split % NPC == 0 and split <= 32767 + 1 and N - split <= 32767 + 1
    return SimpleNamespace(
        N=N, E=E, D=D, H=H, HD=HD, cores=cores, NPC=NPC, NT=NT, NTA=NTA,
        ROWU=ROWU, ROWF=ROWU // 2, SPLIT=split,
        G1=8, GB=7,
        NEG=0.2,
    )


# ---------------------------------------------------------------- host prep

def perm_h_inner(D, H):
    """perm[j] = original feature index stored at packed col j (h-innermost)."""
    HD = D // H
    j = np.arange(D)
    return (j % H) * HD + j // H


def attn_cols(W, a, H):
    D = W.shape[0]
    HD = W.shape[1] // H
    return np.stack(
        [W[:, h * HD:(h + 1) * HD] @ a[h] for h in range(H)], axis=1
    )


def wrap16(flat, reps=8):
    """[num] -> [16*reps, num//16] int16 wrapped layout (idx i at [i%16, i//16]),
    replicated across the 8 gpsimd cores."""
    num = len(flat)
    assert num % 16 == 0
    a = np.zeros((16, num // 16), dtype=np.int16)
    a[np.arange(num) % 16, np.arange(num) // 16] = flat
    return np.tile(a, (reps, 1))


def prep_edges(src, dst, cfg):
    c = cfg
    order = np.argsort(dst, kind="stable")
    src_s = src[order].astype(np.int64)
    dst_s = dst[order].astype(np.int64)

    core = dst_s // c.NPC
    loc = dst_s - core * c.NPC
    lt = loc // P
    pos = loc - lt * P
    islow = src_s < c.SPLIT

    # group = (core, tile, islow) ; edges of a group get consecutive slots
    g = (core * c.NT + lt) * 2 + (1 - islow)   # low first
    order2 = np.argsort(g, kind="stable")
    src_s, dst_s, core, lt, pos, islow, g = (
        a[order2] for a in (src_s, dst_s, core, lt, pos, islow, g))

    uniq, starts = np.unique(g, return_index=True)
    start_of = np.zeros(c.cores * c.NT * 2, dtype=np.int64)
    start_of[uniq] = starts
    rank = np.arange(len(dst_s)) - start_of[g]

    counts = np.zeros(c.cores * c.NT * 2, dtype=np.int64)
    np.add.at(counts, g, 1)
    KA = int(math.ceil(counts[0::2].max() / P))
    KB = int(math.ceil(counts[1::2].max() / P)) if counts[1::2].max() > 0 else 0
    K = KA + KB

    # slot within tile: low edges fill chunks [0,KA), high fill [KA,K)
    chunk = rank // P + np.where(islow, 0, KA)
    part = rank % P

    srcA = np.zeros((c.cores, c.NT * KA * P), dtype=np.int64)
    srcB = np.zeros((c.cores, c.NT * KB * P), dtype=np.int64) if KB else None
    dstpos = np.full((c.cores, P, c.NT * K), 255.0, dtype=np.float32)
    dstposT = np.full((c.cores, c.NT * K * P), 255, dtype=np.int16)

    iA = lt * (KA * P) + (chunk * P + part)            # valid where islow
    iB = lt * (KB * P) + ((chunk - KA) * P + part) if KB else None
    low = islow
    srcA[core[low], iA[low]] = src_s[low]
    if KB:
        hi = ~islow
        srcB[core[hi], iB[hi]] = src_s[hi] - c.SPLIT
    dstpos[core, part, lt * K + chunk] = pos
    dstposT[core, (lt * K + chunk) * P + part] = pos

    # host-built transposed one-hots M[d, (t, ck, p)] = (dstposT == d).
    # fp8e4m3 is exact for 0/1 and halves the stream bytes.
    import ml_dtypes
    Mhot = (dstposT[:, None, :] == np.arange(P, dtype=np.int16)[None, :, None]
            ).astype(ml_dtypes.float8_e4m3fn)

    srcA16 = np.stack([wrap16(srcA[ci]) for ci in range(c.cores)])
    srcB16 = (np.stack([wrap16(srcB[ci]) for ci in range(c.cores)])
              if KB else np.zeros((c.cores, P, 0), np.int16))
    return srcA16, srcB16, dstpos, Mhot, KA, KB


def prep_all(inputs, cfg):
    c = cfg
    perm = perm_h_inner(c.D, c.H)
    x = np.asarray(inputs["data"], np.float32)
    src = np.asarray(inputs["src"]).astype(np.int64)
    dst = np.asarray(inputs["dst"]).astype(np.int64)

    def rhs_for(W, al, ar, permute_rows):
        W = np.asarray(W, np.float64)
        Wal = attn_cols(W, np.asarray(al, np.float64), c.H)
        War = attn_cols(W, np.asarray(ar, np.float64), c.H)
        Wp = W[:, perm]
        if permute_rows:
            Wp, Wal, War = Wp[perm], Wal[perm], War[perm]
        return to_bf16(np.concatenate([Wp, Wal, War], axis=1))

    rhs1 = rhs_for(inputs["W1"], inputs["al1"], inputs["ar1"], False)
    rhs2 = rhs_for(inputs["W2"], inputs["al2"], inputs["ar2"], True)
    b1 = np.asarray(inputs["b1"], np.float32)[perm].reshape(1, c.D)
    b2 = np.asarray(inputs["b2"], np.float32)[perm].reshape(1, c.D)
    xT = to_bf16(x.T.copy())

    srcA16, srcB16, dstpos, Mhot, KA, KB = prep_edges(src, dst, c)

    # er build indices (layer 1): row i (i in [0, NT*128)) <- own node NPC*ci+i
    erbA, erbB, masks = [], [], []
    for ci in range(c.cores):
        base = ci * c.NPC
        rows = np.arange(c.NT * P)
        ra = np.where(base < c.SPLIT, base + rows, 0)
        ra = np.clip(ra, 0, c.SPLIT - 1)
        rb = np.where(base >= c.SPLIT, base - c.SPLIT + rows, 0)
        rb = np.clip(rb, 0, max(c.N - c.SPLIT - 1, 0))
        erbA.append(wrap16(ra))
        erbB.append(wrap16(rb))
        mA = 1.0 if base < c.SPLIT else 0.0
        m = np.zeros((P, 2), np.float32)
        m[:, 0] = mA
        m[:, 1] = 1.0 - mA
        masks.append(m)

    meta = SimpleNamespace(perm=perm, KA=KA, KB=KB, K=KA + KB)
    in_maps = []
    for ci in range(c.cores):
        in_maps.append({
            "xT": xT, "rhs1": rhs1, "rhs2": rhs2, "b1row": b1, "b2row": b2,
            "srcA16": srcA16[ci], "srcB16": srcB16[ci],
            "dstpos": to_bf16(dstpos[ci]),
            "Mhot": Mhot[ci],
            "erbA16": erbA[ci], "erbB16": erbB[ci],
            "maskAB": masks[ci],
        })
    return in_maps, meta


def to_bf16(a):
    import ml_dtypes
    return np.asarray(a).astype(ml_dtypes.bfloat16)


def finalize(results, cfg, meta):
    c = cfg
    parts = [results[ci]["out"][: c.NPC] for ci in range(c.cores)]
    out_p = np.concatenate(parts, axis=0)
    out = np.empty_like(out_p)
    out[:, meta.perm] = out_p
    return out


# ---------------------------------------------------------------- kernel

def build_nc(cfg, KA, KB, debug=False, reps=1):
    c = cfg
    K = KA + KB
    ERR = c.NT * P                   # er rows per core (padded)
    NPAD = c.NTA * P                 # padded table rows (layer 1)
    DH = c.D + c.H                   # seg-matmul rhs cols (msg + den)
    D2H = c.D + 2 * c.H              # phase-A rhs cols
    WC = c.D // 2 + 2 * c.H          # written f32 cols of a packed row

    nc = bacc.Bacc("TRN2", target_bir_lowering=False, debug=debug,
                   num_devices=c.cores, num_swdge_queues=2)

    xT = nc.declare_dram_parameter("xT", [c.D, c.N], BF16, isOutput=False)
    rhs1 = nc.declare_dram_parameter("rhs1", [c.D, D2H], BF16, isOutput=False)
    rhs2 = nc.declare_dram_parameter("rhs2", [c.D, D2H], BF16, isOutput=False)
    b1row = nc.declare_dram_parameter("b1row", [1, c.D], F32, isOutput=False)
    b2row = nc.declare_dram_parameter("b2row", [1, c.D], F32, isOutput=False)
    srcA16 = nc.declare_dram_parameter("srcA16", [P, c.NT * KA * 8], I16, isOutput=False)
    srcB16 = nc.declare_dram_parameter("srcB16", [P, c.NT * KB * 8], I16, isOutput=False)
    dstpos = nc.declare_dram_parameter("dstpos", [P, c.NT * K], BF16, isOutput=False)
    Mhot = nc.declare_dram_parameter("Mhot", [P, c.NT * K * P], FP8, isOutput=False)
    erbA16 = nc.declare_dram_parameter("erbA16", [P, ERR // 16], I16, isOutput=False)
    erbB16 = nc.declare_dram_parameter("erbB16", [P, ERR // 16], I16, isOutput=False)
    maskAB = nc.declare_dram_parameter("maskAB", [P, 2], F32, isOutput=False)
    out_ext = nc.declare_dram_parameter("out", [c.NT * P, c.D], F32, isOutput=True)

    tab1A = nc.dram_tensor("tab1A", [c.SPLIT, c.ROWF], F32)
    tab1B = nc.dram_tensor("tab1B", [NPAD - c.SPLIT, c.ROWF], F32)
    tab2loc = nc.dram_tensor("tab2loc", [c.NPC, c.ROWF], F32)
    tab2 = nc.dram_tensor("tab2", [c.N, c.ROWF], F32,
                          addr_space="Shared" if c.cores > 4 else "Local")

    with tile.TileContext(nc) as tc:
        with (
            tc.tile_pool(name="const", bufs=1) as constp,
            tc.tile_pool(name="slab", bufs=4) as slabp,
            tc.tile_pool(name="pkg", bufs=2) as pkgp,
            tc.tile_pool(name="eb", bufs=2) as ebp,
            tc.tile_pool(name="ertab", bufs=1) as ertabp,
            tc.tile_pool(name="gath", bufs=4) as gathp,
            tc.tile_pool(name="onehot", bufs=3) as onehotp,
            tc.tile_pool(name="sel", bufs=3) as selp,
            tc.tile_pool(name="rhsm", bufs=2) as rhsmp,
            tc.tile_pool(name="small", bufs=3) as smallp,
            tc.tile_pool(name="lt2", bufs=3) as lt2p,
            tc.tile_pool(name="outp", bufs=2) as outp,
            tc.tile_pool(name="psA", bufs=4, space="PSUM") as psA,
            tc.tile_pool(name="psB", bufs=2, space="PSUM") as psB,
            tc.tile_pool(name="psE", bufs=1, space="PSUM") as psE,
            tc.tile_pool(name="psT", bufs=1, space="PSUM") as psT,
        ):
            # ---------------- constants
            iota = constp.tile([P, P], BF16, tag="iota")
            nc.gpsimd.iota(iota[:], [[1, P]], channel_multiplier=0,
                           allow_small_or_imprecise_dtypes=True)
            iota_kp = constp.tile([P, P, K], BF16, tag="iota_kp")
            nc.gpsimd.iota(iota_kp[:], [[1, P], [0, K]], channel_multiplier=0,
                           allow_small_or_imprecise_dtypes=True)
            from concourse.masks import make_identity
            ident = constp.tile([P, P], BF16, tag="ident")
            make_identity(nc, ident[:])

            def load_const(name, param, shape, dt):
                t = constp.tile(shape, dt, tag=name, name=name)
                nc.sync.dma_start(out=t[:], in_=param[:, :])
                return t

            srcA_sb = load_const("srcA_sb", srcA16, [P, c.NT * KA * 8], I16)
            srcB_sb = load_const("srcB_sb", srcB16, [P, c.NT * KB * 8], I16)
            dstpos_sb = load_const("dstpos_sb", dstpos, [P, c.NT * K], BF16)
            erbA_sb = load_const("erbA_sb", erbA16, [P, ERR // 16], I16)
            erbB_sb = load_const("erbB_sb", erbB16, [P, ERR // 16], I16)
            mask_sb = load_const("mask_sb", maskAB, [P, 2], F32)

            rhsW = [[constp.tile([P, D2H], BF16,
                                 tag=f"rhsW{l}_{kb}", name=f"rhsW{l}_{kb}")
                     for kb in range(2)] for l in range(2)]
            for l, rt in enumerate([rhs1, rhs2]):
                for kb in range(2):
                    nc.sync.dma_start(out=rhsW[l][kb][:],
                                      in_=rt[kb * P: (kb + 1) * P, :])
            b_bc = [constp.tile([P, c.D], F32, tag=f"bbc{l}", name=f"bbc{l}")
                    for l in range(2)]
            for l, bt in enumerate([b1row, b2row]):
                nc.sync.dma_start(out=b_bc[l][:],
                                  in_=bt[0:1, :].to_broadcast([P, c.D]))

            # Pre-touch consts on compute engines so first uses don't carry
            # extra sync waits on deep pipelines.
            warm = constp.tile([P, 4], F32, tag="warm")
            warmb = warm[:].bitcast(BF16)
            nc.vector.tensor_copy(out=warm[:, 0:1], in_=dstpos_sb[:, 0:1])
            nc.vector.tensor_copy(out=warmb[:, 2:3], in_=iota[:, 0:1])
            nc.vector.tensor_copy(out=warm[:, 1:2], in_=mask_sb[:, 0:1])

            er_tab = [ertabp.tile([P, c.NT, c.H], BF16, tag=f"ertab{l}",
                                  name=f"ertab{l}")
                      for l in range(2)]

            # ---------------- phase A layer 1 (replicated, batched groups)
            def phase_a1():
                ngrp = math.ceil(c.NTA / c.G1)
                for grp in range(ngrp):
                    t0 = grp * c.G1
                    gt = min(c.G1, c.NTA - t0)
                    cols = min(gt * P, c.N - t0 * P)
                    slab = slabp.tile([P, 2, c.G1 * P], BF16, tag="slab")
                    nc.scalar.dma_start(
                        out=slab[:, :, :cols],
                        in_=xT[:, t0 * P: t0 * P + cols]
                        .rearrange("(kb p) n -> p kb n", p=P))
                    pkg = pkgp.tile([P, c.G1, c.ROWF], F32, tag="pkg")
                    pkgb = pkg[:].bitcast(BF16)
                    for j in range(gt):
                        m = min(P, c.N - (t0 + j) * P)
                        ps = psA.tile([P, D2H], F32, tag="psA")
                        for kb in range(2):
                            nc.tensor.matmul(
                                out=ps[:m, :],
                                lhsT=slab[:, kb, j * P: j * P + m],
                                rhs=rhsW[0][kb][:],
                                start=(kb == 0), stop=(kb == 1))
                        if j % 2 == 0:
                            nc.scalar.copy(out=pkgb[:m, j, : c.D],
                                           in_=ps[:m, : c.D])
                            nc.vector.tensor_copy(
                                out=pkg[:m, j, c.D // 2: WC],
                                in_=ps[:m, c.D: D2H])
                        else:
                            nc.vector.tensor_copy(out=pkgb[:m, j, : c.D],
                                                  in_=ps[:m, : c.D])
                            nc.scalar.copy(
                                out=pkg[:m, j, c.D // 2: WC],
                                in_=ps[:m, c.D: D2H])
                    r0 = t0 * P
                    rend = (t0 + gt) * P
                    pieces = []
                    for j in range(gt):
                        a = r0 + j * P
                        for dstT, lo_r, hi_r, base in (
                                (tab1A, 0, c.SPLIT, 0),
                                (tab1B, c.SPLIT, NPAD, c.SPLIT)):
                            s = max(a, lo_r)
                            e = min(a + P, hi_r)
                            if s < e:
                                pieces.append([dstT, s - base, j, s - a, e - s])
                    runs = []
                    for pc in pieces:
                        if (runs and pc[3] == 0 and pc[4] == P
                                and runs[-1][0] is pc[0]
                                and runs[-1][4] == P and runs[-1][3] == 0
                                and runs[-1][1] + runs[-1][5] * P == pc[1]
                                and runs[-1][2] + runs[-1][5] == pc[2]):
                            runs[-1][5] += 1
                        else:
                            runs.append(pc + [1 if pc[4] == P and pc[3] == 0
                                              else 0])
                    for dstT, drow0, j0, p0, n, nj in runs:
                        if nj:
                            nc.sync.dma_start(
                                out=dstT[drow0: drow0 + nj * P, :WC]
                                .rearrange("(j p) w -> p j w", p=P),
                                in_=pkg[:, j0: j0 + nj, :WC])
                        else:
                            nc.sync.dma_start(
                                out=dstT[drow0: drow0 + n, :WC],
                                in_=pkg[p0: p0 + n, j0, :WC])

            # ---------------- er_tab build for layer 1 (2 gathers + merge)
            def build_er1():
                start = c.ROWF - 64
                eroff = (c.D // 2 + c.H) - start
                half_nt = (c.NT + 1) // 2
                for piece in range(2):
                    tlo = piece * half_nt
                    tn = min(half_nt, c.NT - tlo)
                    ebs = []
                    for half, (tabh, idx_sb) in enumerate([
                            (tab1A, erbA_sb),
                            (tab1B, erbB_sb)]):
                        eb = ebp.tile([P, half_nt, 64], F32,
                                      tag=f"eb{half}", name=f"eb{half}")
                        nc.gpsimd.dma_gather(
                            out_ap=eb[:, :tn, :],
                            in_ap=tabh[:, start: start + 64],
                            idxs_ap=idx_sb[:, tlo * 8: (tlo + tn) * 8],
                            num_idxs=tn * P, num_idxs_reg=tn * P,
                            elem_size=64, elem_step=c.ROWF,
                            single_packet=False, queue_num=half)
                        ebs.append(eb)
                    tmp = smallp.tile([P, half_nt, c.H], F32, tag="ermerge")
                    nc.vector.tensor_scalar(
                        out=tmp[:, :tn], in0=ebs[0][:, :tn, eroff:eroff + c.H],
                        scalar1=mask_sb[:, 0:1], scalar2=None,
                        op0=mybir.AluOpType.mult)
                    nc.vector.scalar_tensor_tensor(
                        out=er_tab[0][:, tlo: tlo + tn],
                        in0=ebs[1][:, :tn, eroff:eroff + c.H],
                        scalar=mask_sb[:, 1:2], in1=tmp[:, :tn],
                        op0=mybir.AluOpType.mult, op1=mybir.AluOpType.add)

            # ---------------- phase B (layer = 0 or 1)
            def phase_b(layer):
                pass
                ngrp = math.ceil(c.NT / c.GB)
                for grp in range(ngrp):
                    t0 = grp * c.GB
                    gt = min(c.GB, c.NT - t0)
                    if layer == 0:
                        pkg2 = pkgp.tile([P, c.GB, c.ROWF], F32, tag="pkg2")
                        pkg2b = pkg2[:].bitcast(BF16)
                    else:
                        o2g = outp.tile([P, c.GB, c.D], F32, tag="o2g")
                    for j in range(gt):
                        t = t0 + j
                        g = gathp.tile([P, K, c.ROWF], F32, tag="gath")
                        nc.gpsimd.dma_gather(
                            out_ap=g[:, 0:KA, :],
                            in_ap=(tab1A if layer == 0 else tab2)[0: c.SPLIT, :],
                            idxs_ap=srcA_sb[:, t * KA * 8:(t + 1) * KA * 8],
                            num_idxs=KA * P, num_idxs_reg=KA * P,
                            elem_size=c.ROWF, single_packet=KA * P <= 1024)
                        nc.gpsimd.dma_gather(
                            out_ap=g[:, KA:K, :],
                            in_ap=(tab1B[0: c.N - c.SPLIT, :] if layer == 0
                                   else tab2[c.SPLIT: c.N, :]),
                            idxs_ap=srcB_sb[:, t * KB * 8:(t + 1) * KB * 8],
                            num_idxs=KB * P, num_idxs_reg=KB * P,
                            elem_size=c.ROWF, single_packet=KB * P <= 1024,
                            queue_num=1)
                        gb = g[:].bitcast(BF16)

                        # transposed one-hots for this tile (host data)
                        M_all = onehotp.tile([P, K, P], FP8, tag="M")
                        nc.scalar.dma_start(
                            out=M_all[:],
                            in_=Mhot[:, t * K * P: (t + 1) * K * P]
                            .rearrange("p (k q) -> p k q", k=K))

                        # er per slot: K tiny matmuls M_all[:, ck, :]^T @ er_tab
                        erp = psE.tile([P, K * c.H], F32, tag="psE")
                        for ck in range(K):
                            nc.tensor.matmul(
                                out=erp[:, ck * c.H:(ck + 1) * c.H],
                                lhsT=M_all[:, ck, :],
                                rhs=er_tab[layer][:, t, :],
                                start=True, stop=True)

                        # e = leakyrelu(el + er); exp
                        ea = smallp.tile([P, K, c.H], F32, tag="eadd")
                        nc.vector.tensor_tensor(
                            out=ea[:], in0=g[:, :, c.D // 2: c.D // 2 + c.H],
                            in1=erp[:].rearrange("p (k h) -> p k h", h=c.H),
                            op=mybir.AluOpType.add)
                        lr = smallp.tile([P, K, c.H], F32, tag="lrout")
                        nc.vector.scalar_tensor_tensor(
                            out=lr[:], in0=ea[:], scalar=c.NEG, in1=ea[:],
                            op0=mybir.AluOpType.mult, op1=mybir.AluOpType.max)
                        rm = rhsmp.tile([P, K, DH], BF16, tag="rhsm")
                        nc.scalar.activation(
                            out=rm[:, :, c.D: DH], in_=lr[:],
                            func=mybir.ActivationFunctionType.Exp)
                        expb = rm[:, :, c.D: DH].unsqueeze(2).to_broadcast(
                            [P, K, c.HD, c.H])
                        feat4 = gb[:, :, : c.D].rearrange(
                            "p k (hd h) -> p k hd h", h=c.H)
                        out4 = rm[:, :, : c.D].rearrange(
                            "p k (hd h) -> p k hd h", h=c.H)
                        nc.vector.tensor_tensor(out=out4, in0=feat4, in1=expb,
                                                op=mybir.AluOpType.mult)

                        # segment sum via one-hot matmuls; all K S-chunks
                        # generated in one 2x-eligible DVE op (layout [P,P,K])
                        S_allT = selp.tile([P, P, K], BF16, tag="S_allT")
                        nc.vector.tensor_tensor(
                            out=S_allT[:],
                            in0=iota_kp[:],
                            in1=dstpos_sb[:, t * K:(t + 1) * K]
                            .unsqueeze(1).to_broadcast([P, P, K]),
                            op=mybir.AluOpType.is_equal)
                        ps = psB.tile([P, DH], F32, tag="psB")
                        for ck in range(K):
                            nc.tensor.matmul(out=ps[:], lhsT=S_allT[:, :, ck],
                                             rhs=rm[:, ck, :],
                                             start=(ck == 0), stop=(ck == K - 1))
                        den = smallp.tile([P, c.H], F32, tag="den")
                        nc.vector.tensor_scalar_max(den[:], ps[:, c.D: DH], 1e-30)
                        rcp = smallp.tile([P, c.H], F32, tag="rcp")
                        nc.vector.reciprocal(rcp[:], den[:])
                        o1 = outp.tile([P, c.D], F32, tag="o1")
                        rcpb = rcp[:].unsqueeze(1).to_broadcast([P, c.HD, c.H])
                        ps4 = ps[:, : c.D].rearrange("p (hd h) -> p hd h", h=c.H)
                        o14 = o1[:].rearrange("p (hd h) -> p hd h", h=c.H)
                        nc.vector.tensor_tensor(out=o14, in0=ps4, in1=rcpb,
                                                op=mybir.AluOpType.mult)
                        nc.vector.tensor_tensor(out=o1[:], in0=o1[:],
                                                in1=b_bc[layer][:],
                                                op=mybir.AluOpType.add)
                        if layer == 0:
                            # h for this tile -> layer-2 phase A (sharded)
                            hb = outp.tile([P, c.D], BF16, tag="hb")
                            nc.vector.tensor_scalar_max(hb[:], o1[:], 0.0)
                            ps2 = psA.tile([P, D2H], F32, tag="psA")
                            for kb in range(2):
                                pst = psT.tile([P, P], BF16, tag="psT")
                                nc.tensor.transpose(
                                    out=pst[:],
                                    in_=hb[:, kb * P: (kb + 1) * P],
                                    identity=ident[:])
                                lt2 = lt2p.tile([P, P], BF16, tag="lt2")
                                nc.scalar.copy(out=lt2[:], in_=pst[:])
                                nc.tensor.matmul(
                                    out=ps2[:], lhsT=lt2[:],
                                    rhs=rhsW[1][kb][:],
                                    start=(kb == 0), stop=(kb == 1))
                            nc.scalar.copy(out=pkg2b[:, j, : c.D],
                                           in_=ps2[:, : c.D])
                            nc.scalar.copy(
                                out=pkg2[:, j, c.D // 2: c.D // 2 + c.H],
                                in_=ps2[:, c.D: c.D + c.H])
                            nc.scalar.copy(
                                out=er_tab[1][:, t, :],
                                in_=ps2[:, c.D + c.H: D2H])
                        else:
                            nc.vector.tensor_scalar_max(o2g[:, j, :], o1[:], 0.0)
                    # group epilogue
                    if layer == 0:
                        rows = min(gt * P, c.NPC - t0 * P)
                        full = rows // P
                        WEL = c.D // 2 + c.H
                        if full:
                            nc.sync.dma_start(
                                out=tab2loc[t0 * P: t0 * P + full * P, :WEL]
                                .rearrange("(j p) w -> p j w", p=P),
                                in_=pkg2[:, :full, :WEL])
                        tail = rows - full * P
                        if tail:
                            nc.sync.dma_start(
                                out=tab2loc[t0 * P + full * P:
                                            t0 * P + full * P + tail, :WEL],
                                in_=pkg2[:tail, full, :WEL])
                    else:
                        nc.sync.dma_start(
                            out=out_ext[t0 * P: (t0 + gt) * P, :]
                            .rearrange("(j p) d -> p j d", p=P),
                            in_=o2g[:, :gt, :])

            for _rep in range(reps):
                phase_a1()
                build_er1()
                phase_b(0)
                nc.gpsimd.collective_compute(
                    "AllGather",
                    mybir.AluOpType.bypass,
                    replica_groups=[list(range(c.cores))],
                    ins=[tab2loc[:]],
                    outs=[tab2[:]],
                )
                phase_b(1)

    nc.compile()
    return nc


# ---------------------------------------------------------------- reference

def ref_np(inputs, cfg):
    c = cfg
    x = np.asarray(inputs["data"], np.float64)
    src = np.asarray(inputs["src"]).astype(np.int64)
    dst = np.asarray(inputs["dst"]).astype(np.int64)

    def layer(x, W, al, ar, b):
        N = x.shape[0]
        feat = (x @ np.asarray(W, np.float64)).reshape(N, c.H, c.HD)
        el = np.einsum("nhd,hd->nh", feat, np.asarray(al, np.float64))
        er = np.einsum("nhd,hd->nh", feat, np.asarray(ar, np.float64))
        e = el[src] + er[dst]
        e = np.where(e > 0, e, c.NEG * e)
        m = np.full((N, c.H), -np.inf)
        np.maximum.at(m, dst, e)
        a = np.exp(e - m[dst])
        den = np.zeros((N, c.H))
        np.add.at(den, dst, a)
        alpha = a / den[dst]
        msg = feat[src] * alpha[:, :, None]
        out = np.zeros((N, c.H, c.HD))
        np.add.at(out, dst, msg)
        out = out + np.asarray(b, np.float64).reshape(1, c.H, c.HD)
        return np.maximum(out, 0).reshape(N, c.D)

    h = layer(x, inputs["W1"], inputs["al1"], inputs["ar1"], inputs["b1"])
    h = layer(h, inputs["W2"], inputs["al2"], inputs["ar2"], inputs["b2"])
    return h


# ---------------------------------------------------------------- entry point

_BUILD_CACHE = {}


def kernel(**inputs) -> np.ndarray:
    """Full-input GAT kernel: shards internally across 8 NeuronCores."""
    from concourse.bass_utils import run_bass_kernel_spmd

    cfg = make_cfg(N=50000, E=800000, D=256, H=8, cores=8)
    in_maps, meta = prep_all(inputs, cfg)
    key = (meta.KA, meta.KB)
    if key not in _BUILD_CACHE:
        _BUILD_CACHE[key] = build_nc(cfg, meta.KA, meta.KB)
    nc = _BUILD_CACHE[key]
    res = run_bass_kernel_spmd(nc, in_maps, list(range(cfg.cores)))
    results = [{"out": res.results[ci]["out"]} for ci in range(cfg.cores)]
    out = finalize(results, cfg, meta)
    return np.ascontiguousarray(out.astype(np.float32))
